# revision 5
# baseline (speedup 1.0000x reference)
"""Trainium2 Bass kernel for local cost-volume correlation (FlowNet-style), v3.

Problem: in1, in2 [B=8, C=256, H=96, W=128] fp32; out [B, 81, H, W] where
out[b, dy*9+dx, h, w] = mean_c in1[b,c,h,w] * in2[b,c,h+dy-4,w+dx-4] (zero pad).

Sharding: data-parallel over batch, one image per NeuronCore (8 cores).

v3 design (vs v2): R=8 row-packing, bias+max select, batched DMAs.
  - Host: in1, in2 scaled by 1/16 each (mean folded in) and cast to f16.
    Output f16, upcast on host.
  - Weight tile per (k, g): 8 in1 rows x 16 pixels (p = delta*16 + w2);
    rhs streams 16 in2 ring rows x 24-u window (N=384).  Each in2 row is
    streamed twice per k instead of 9x: PE cols/row = 2*384/8*8 = 768+tr.
  - in2 ring [P, 43, CK, 140] f16: slot(r) = (r+4) % 28, slots 0..14
    mirrored at +28 so any 16-slot window is contiguous.  One DMA per
    4-row quad (+1 mirror DMA), one iteration (8 rows) ahead.
  - Extraction per iter: psum[p=(delta,w2), j*24+u] --ACT--> S f16
    -> gather1 (3x8 chunks at g*48+(delta+dy)*3, delta = q//2... q=delta)
    -> mod-16 -> stream_shuffle (w2_3 <-> delta_0) -> gather2 (2 chunks,
    base + q%2) -> mod-8 -> ONE bias-add (0 at e==p%8 else -60000) +
    ONE max-reduce over the 8-window -> O[p, (g,dy,dx)].
  - 8 PE transposes (identity = inverse-shuffle perm), lagged one
    iteration; one ACT staging copy; one out-DMA per iter.
"""

import threading

import numpy as np

B, C, H, W = 8, 256, 96, 128
ND = 9             # displacement range per axis
NCH = ND * ND      # 81 output channels
CK = 2             # C // 128 contraction chunks
P = 128
RPI = 8            # rows per iteration
NT = 8             # pixel-group tiles per iteration (16 pixels each)
G = 16             # pixels per tile
JW = 16            # in2 row window per iteration (h-4 .. h+11)
UW = 24            # per-tile u window
BANDT = JW * UW    # 384, gram band per tile
RD = NT * ND       # 72 (g, dy) pairs per iteration
NSLOTP = 28        # physical ring slots
NMIR = 12          # slots 0..11 mirrored at +28
NSLOT = NSLOTP + NMIR  # 40
SROW = 140         # padded in2 row width (4 + 128 + 8)
G1D = 8            # gather chunk width
G1N = 112          # gather1 num_idxs per half-iteration (108 real + pad)
G2N = 160          # gather2 num_idxs (144 real + pad; pad also gives the
                   # +1-shifted q8b read one element of slack)
SCALE = 0.0625     # host-side per-input scale; SCALE^2 = 1/C

# stream_shuffle mask: swap bit3 (w2_3) with bit4 (delta_0) within each
# 32-partition quadrant
SHUF = list(range(0, 8)) + list(range(16, 24)) + list(range(8, 16)) + list(range(24, 32))

_cache = {}
_lock = threading.Lock()


def _wrap_idx(flat, ncols):
    n = flat.shape[1]
    out = np.zeros((P, ncols), dtype=np.int16)
    for q in range(8):
        for i in range(n):
            out[16 * q + (i % 16), i // 16] = flat[q, i]
    return out


def _host_tables():
    # gather1: S viewed [128, 384, 8]; for (g, dy) gather the whole 24-elem
    # u-window: 3 chunks at g*48 + (delta+dy)*3; delta = q.  -> mod-16.
    flat1 = np.zeros((8, G1N), dtype=np.int16)
    for q in range(8):
        delta = q
        for i in range(G1N):
            g, rem = divmod(min(i, 107), 27)
            dy, c = divmod(rem, 3)
            flat1[q, i] = g * 48 + (delta + dy) * 3 + c
    g1 = _wrap_idx(flat1, 7)

    # gather2: qs viewed [128, 216, 8]; for (g, dy) gather 2 chunks at
    # (g*9+dy)*3 + q%2 (q%2 = w2_3 after the shuffle).  -> mod-8.
    flat2 = np.zeros((8, G2N), dtype=np.int16)
    for q in range(8):
        for i in range(RD * 2):
            gd, c = divmod(i, 2)
            flat2[q, i] = gd * 3 + (q % 2) + c
    g2 = _wrap_idx(flat2, G2N // 16)

    # fully-materialized one-hot masks (no stride-0 dims -> DVE 2x mode)
    me = np.zeros((P, RD, 5, 8), dtype=np.float16)
    mo = np.zeros((P, RD, 4, 8), dtype=np.float16)
    for p in range(P):
        me[p, :, :, p % 8] = 1.0
        mo[p, :, :, p % 8] = 1.0
    # inverse shuffle permutation matrix: perm[p_new, old(p_new)] = 1
    perm = np.zeros((P, P), dtype=np.float16)
    for s in range(4):
        for i in range(32):
            perm[32 * s + i, 32 * s + SHUF[i]] = 1.0
    return g1, g2, me.reshape(P, -1), mo.reshape(P, -1), perm


def _build_nc():
    from contextlib import ExitStack

    import concourse.bass as bass
    import concourse.mybir as mybir
    import concourse.tile as tile
    from concourse import bacc

    f32 = mybir.dt.float32
    f16 = mybir.dt.float16
    i16 = mybir.dt.int16
    u32 = mybir.dt.uint32

    nc = bacc.Bacc("TRN2", target_bir_lowering=False, debug=False)
    # in1 pre-arranged on host to [p, hb, k, g, delta*16+w2] so each weight
    # tile (k, g) is a contiguous 128-column slab (walrus requires weight
    # APs to have a single free dimension).
    in1 = nc.declare_dram_parameter(
        "in1", [P, H // RPI, CK, NT, P], f16, isOutput=False
    )
    in2 = nc.declare_dram_parameter("in2", [C, H, W], f16, isOutput=False)
    g1t = nc.declare_dram_parameter("g1t", [P, 7], i16, isOutput=False)
    g2t = nc.declare_dram_parameter("g2t", [P, G2N // 16], i16, isOutput=False)
    met = nc.declare_dram_parameter("met", [P, RD * 40], f16, isOutput=False)
    mot = nc.declare_dram_parameter("mot", [P, RD * 32], f16, isOutput=False)
    permt = nc.declare_dram_parameter("permt", [P, P], f16, isOutput=False)
    out_t = nc.declare_dram_parameter("out", [NCH, H, W], f16, isOutput=True)

    # in2 accessed (h, k, w) to match the ring's [slot, k, u] layout
    in2r = in2[:].rearrange("(k p) h w -> p h k w", p=P)

    with ExitStack() as ctx:
        tc = ctx.enter_context(tile.TileContext(nc))
        const = ctx.enter_context(tc.tile_pool(name="const", bufs=1))
        persist = ctx.enter_context(tc.tile_pool(name="persist", bufs=1))
        inp = ctx.enter_context(tc.tile_pool(name="inp", bufs=3))
        sp = ctx.enter_context(tc.tile_pool(name="sp", bufs=3))
        qp = ctx.enter_context(tc.tile_pool(name="qp", bufs=3))
        qsp = ctx.enter_context(tc.tile_pool(name="qsp", bufs=3))
        q8p = ctx.enter_context(tc.tile_pool(name="q8p", bufs=4))
        pmp = ctx.enter_context(tc.tile_pool(name="pmp", bufs=2))
        op = ctx.enter_context(tc.tile_pool(name="op", bufs=4))
        stg = ctx.enter_context(tc.tile_pool(name="stg", bufs=3))
        gram = ctx.enter_context(tc.tile_pool(name="gram", bufs=6, space="PSUM"))
        ptp = ctx.enter_context(tc.tile_pool(name="ptp", bufs=2, space="PSUM"))

        g1_s = const.tile([P, 7], i16)
        nc.sync.dma_start(out=g1_s[:], in_=g1t[:])
        g2_s = const.tile([P, G2N // 16], i16)
        nc.sync.dma_start(out=g2_s[:], in_=g2t[:])
        me_s = const.tile([P, RD * 40], f16)
        nc.sync.dma_start(out=me_s[:], in_=met[:])
        mo_s = const.tile([P, RD * 32], f16)
        nc.sync.dma_start(out=mo_s[:], in_=mot[:])
        perm_s = const.tile([P, P], f16)
        nc.sync.dma_start(out=perm_s[:], in_=permt[:])

        # in2 ring: [p, slot, k, u]; slot(r) = (r+4) % 28, mirrored at +28
        # for slots 0..14 so the 16-slot j-window is always contiguous.
        Rr = persist.tile([P, NSLOT, CK, SROW], f16)
        # zero only what DMA never writes: the pad columns and the
        # initial rows<0 slots (0..3 and mirrors)
        nc.gpsimd.memset(Rr[:, :, :, 0:4], 0.0)
        nc.gpsimd.memset(Rr[:, :, :, 4 + W : SROW], 0.0)
        nc.gpsimd.memset(Rr[:, 0:4, :, 4 : 4 + W], 0.0)
        nc.gpsimd.memset(Rr[:, NSLOTP : NSLOTP + 4, :, 4 : 4 + W], 0.0)

        def ring_wr(s, k, r, n):
            Rra = Rr[:]
            dst = bass.AP(
                tensor=Rra.tensor,
                offset=Rra.offset + s * (CK * SROW) + k * SROW + 4,
                ap=[Rra.ap[0], [CK * SROW, n], [1, W]],
            )
            nc.sync.dma_start(out=dst, in_=in2r[:, r : r + n, k, :])

        def load_in2_8rows(r):
            s = (r + 4) % NSLOTP
            n1 = min(8, NSLOTP - s)  # rows before wrapping past slot 27
            for k in range(CK):
                ring_wr(s, k, r, n1)
                if n1 < 8:
                    ring_wr(0, k, r + n1, 8 - n1)
                # mirror rows landing at slots 0..NMIR-1
                if s < NMIR:
                    ring_wr(s + NSLOTP, k, r, min(n1, NMIR - s))
                if n1 < 8:
                    ring_wr(NSLOTP, k, r + n1, 8 - n1)

        def zero_in2_quads(r):
            # rows r..r+7 are past the image: zero their slots (never in
            # the mirror range for our h grid)
            s = (r + 4) % NSLOTP
            nc.gpsimd.memset(Rr[:, s : s + 8, :, 4 : 4 + W], 0.0)

        win_tiles = {}

        def load_win(hh):
            t = inp.tile([P, CK, NT, P], f16, tag="win")
            nc.scalar.dma_start(out=t[:], in_=in1[:][:, hh // RPI])
            win_tiles[hh] = t

        load_in2_8rows(0)
        load_in2_8rows(8)
        load_win(0)
        load_win(8)

        def emit_output(O, h0):
            # 8 transposes into one psum tile; identity = inverse shuffle
            # permutation so columns land at true (delta, w2).
            pt = ptp.tile([P, NT, P], f16, tag="pt")
            for g in range(NT):
                nc.tensor.transpose(
                    out=pt[0:NCH, g, :],
                    in_=O[:, g * NCH : (g + 1) * NCH],
                    identity=perm_s[:],
                )
            Ost = stg.tile([NCH, RPI, W], f16)
            pta = pt[0:NCH]
            # pt[ch, g, (delta, w2)] -> Ost[ch, delta, g*16 + w2]
            src = bass.AP(
                tensor=pta.tensor,
                offset=pta.offset,
                ap=[pta.ap[0], [G, RPI], [P, NT], [1, G]],
            )
            Oa = Ost[:]
            dst = bass.AP(
                tensor=Oa.tensor,
                offset=Oa.offset,
                ap=[Oa.ap[0], [W, RPI], [G, NT], [1, G]],
            )
            nc.scalar.copy(out=dst, in_=src)
            nc.scalar.dma_start(out=out_t[:, h0 : h0 + RPI, :], in_=Ost[:])

        def run_tree(q8):
            # masked select: value for (p, rd, dxi) at q8[p, rd*16 + dxi + p%8].
            # Even dxi read q8 directly (4B-aligned); odd dxi read a +1-shifted
            # copy so their windows start even too -> everything runs 2x.
            q8a = q8[:].rearrange("p a b -> p (a b)")
            q8b = q8p.tile([P, RD * 16], f16, name="q8b", tag="q8b")
            shift_src = bass.AP(
                tensor=q8a.tensor, offset=q8a.offset + 1,
                ap=[q8a.ap[0], [1, RD * 16]],
            )
            nc.vector.tensor_copy(out=q8b[:], in_=shift_src)

            in_e = bass.AP(
                tensor=q8a.tensor, offset=q8a.offset,
                ap=[q8a.ap[0], [16, RD], [2, 5], [1, 8]],
            )
            q8ba = q8b[:]
            in_o = bass.AP(
                tensor=q8ba.tensor, offset=q8ba.offset,
                ap=[q8ba.ap[0], [16, RD], [2, 4], [1, 8]],
            )
            pe_ = pmp.tile([P, RD, 5, 8], f16, name="pe", tag="pe")
            po_ = pmp.tile([P, RD, 4, 8], f16, name="po", tag="po")
            nc.vector.tensor_mul(
                pe_[:], in_e,
                me_s[:].rearrange("p (a b c) -> p a b c", b=5, c=8),
            )
            nc.vector.tensor_mul(
                po_[:], in_o,
                mo_s[:].rearrange("p (a b c) -> p a b c", b=4, c=8),
            )

            t1e = op.tile([P, RD, 5, 4], f16, name="t1e", tag="t1e")
            t1o = op.tile([P, RD, 4, 4], f16, name="t1o", tag="t1o")
            nc.vector.tensor_add(t1e[:], pe_[:, :, :, 0:4], pe_[:, :, :, 4:8])
            nc.vector.tensor_add(t1o[:], po_[:, :, :, 0:4], po_[:, :, :, 4:8])
            t2e = op.tile([P, RD, 5, 2], f16, name="t2e", tag="t2e")
            t2o = op.tile([P, RD, 4, 2], f16, name="t2o", tag="t2o")
            nc.vector.tensor_add(t2e[:], t1e[:, :, :, 0:2], t1e[:, :, :, 2:4])
            nc.vector.tensor_add(t2o[:], t1o[:, :, :, 0:2], t1o[:, :, :, 2:4])

            O = op.tile([P, RD * ND], f16, name="O", tag="O")
            Oa = O[:]
            out_e = bass.AP(
                tensor=Oa.tensor, offset=Oa.offset,
                ap=[Oa.ap[0], [ND, RD], [2, 5]],
            )
            out_o = bass.AP(
                tensor=Oa.tensor, offset=Oa.offset + 1,
                ap=[Oa.ap[0], [ND, RD], [2, 4]],
            )
            nc.vector.tensor_add(out_e, t2e[:, :, :, 0], t2e[:, :, :, 1])
            nc.vector.tensor_add(out_o, t2o[:, :, :, 0], t2o[:, :, :, 1])
            return O

        pend_tree = None  # (q8, h): select tree lagged one iteration
        pend_emit = []    # [(O, h)]: transposes/staging lagged two
        for h in range(0, H, RPI):
            # prefetch in2 rows h+16..h+23 (one iteration ahead)
            if h + 16 < H:
                load_in2_8rows(h + 16)
            elif h + 16 == H:
                zero_in2_quads(h + 16)
            if h + 16 < H:
                load_win(h + 16)

            win = win_tiles.pop(h)
            wa = win[:]

            # R=8 packed matmuls: per tile g, weights = 8 rows x 16 pixels
            # (p = delta*16 + w2), rhs = 16 ring rows x 24-u window, N=384.
            s0 = h % NSLOTP
            pss = []
            for g in range(NT):
                ps = gram.tile([P, BANDT], f32, tag="ps")
                for k in range(CK):
                    lhsT = bass.AP(
                        tensor=wa.tensor,
                        offset=wa.offset + (k * NT + g) * P,
                        ap=[wa.ap[0], [1, P]],
                    )
                    rhs_t = Rr[:]
                    rhs = bass.AP(
                        tensor=rhs_t.tensor,
                        offset=rhs_t.offset
                        + s0 * (CK * SROW)
                        + k * SROW
                        + G * g,
                        ap=[rhs_t.ap[0], [CK * SROW, JW], [1, UW]],
                    )
                    nc.tensor.matmul(
                        out=ps[:],
                        lhsT=lhsT,
                        rhs=rhs,
                        start=(k == 0),
                        stop=(k == CK - 1),
                    )
                pss.append(ps)

            S = sp.tile([P, NT, BANDT], f16)
            for g in range(NT):
                nc.scalar.copy(out=S[:, g, :], in_=pss[g][:])

            # gather1 -> mod-16 (whole 24-elem windows; j = delta+dy select).
            # Two half-gathers (4 tiles each, same table) so the first can
            # start as soon as the first four band copies land; data moved
            # as uint32 pairs: same 16-byte chunks, half the gpsimd cost.
            q16 = qp.tile([P, 2 * G1N, G1D], f16)
            q16f = q16[:].rearrange("p a b -> p (a b)")
            for half in range(2):
                nc.gpsimd.ap_gather(
                    out_ap=q16f[:, half * G1N * G1D : (half + 1) * G1N * G1D]
                    .bitcast(u32)
                    .rearrange("p (n d) -> p n d", d=G1D // 2),
                    in_ap=S[:, 4 * half : 4 * half + 4, :]
                    .rearrange("p a b -> p (a b)")
                    .bitcast(u32)
                    .rearrange("p (n d) -> p n d", d=G1D // 2),
                    idxs_ap=g1_s[:],
                    channels=P,
                    num_elems=NT * BANDT // (2 * G1D),
                    d=G1D // 2,
                    num_idxs=G1N,
                )

            # run the PREVIOUS iteration's select tree first: its inputs are
            # ready, so DVE stays busy while Pool finishes gather1 above.
            if pend_tree is not None:
                q8_p, h_p = pend_tree
                pend_emit.append((run_tree(q8_p), h_p))
                pend_tree = None

            qs = qsp.tile([P, RD * 24], f16)
            for half in range(2):
                nc.vector.stream_shuffle(
                    out=qs[:, half * 864 : (half + 1) * 864].bitcast(u32),
                    in_=q16f[:, half * G1N * G1D : half * G1N * G1D + 864].bitcast(
                        u32
                    ),
                    mask=SHUF,
                )

            # gather2 -> mod-8 (uint32 pairs again)
            q8 = q8p.tile([P, G2N, G1D], f16)
            nc.gpsimd.ap_gather(
                out_ap=q8[:].rearrange("p a b -> p (a b)").bitcast(u32).rearrange(
                    "p (n d) -> p n d", d=G1D // 2
                ),
                in_ap=qs[:].bitcast(u32).rearrange("p (n d) -> p n d", d=G1D // 2),
                idxs_ap=g2_s[:],
                channels=P,
                num_elems=RD * 24 // G1D,
                d=G1D // 2,
                num_idxs=G2N,
            )

            pend_tree = (q8, h)

            if len(pend_emit) >= 2:
                emit_output(*pend_emit.pop(0))

        if pend_tree is not None:
            q8_p, h_p = pend_tree
            pend_emit.append((run_tree(q8_p), h_p))
        for item in pend_emit:
            emit_output(*item)

    nc.finalize()
    return nc


def _get_nc():
    with _lock:
        if "nc" not in _cache:
            _cache["nc"] = _build_nc()
        return _cache["nc"]


def _in_maps(in1: np.ndarray, in2: np.ndarray):
    g1, g2, me, mo, perm = _host_tables()
    in1 = (np.ascontiguousarray(in1) * SCALE).astype(np.float16)
    in2 = (np.ascontiguousarray(in2) * SCALE).astype(np.float16)
    # [b, c=(k p), h=(hb delta), w=(g w2)] -> [b, p, hb, k, g, (delta w2)]
    in1 = np.ascontiguousarray(
        in1.reshape(B, CK, P, H // RPI, RPI, NT, G).transpose(0, 2, 3, 1, 5, 4, 6)
    ).reshape(B, P, H // RPI, CK, NT, P)
    return [
        {
            "in1": in1[b],
            "in2": in2[b],
            "g1t": g1,
            "g2t": g2,
            "met": me,
            "mot": mo,
            "permt": perm,
        }
        for b in range(B)
    ]


def kernel(in1: np.ndarray, in2: np.ndarray) -> np.ndarray:
    from concourse.bass_utils import run_bass_kernel_spmd

    nc = _get_nc()
    in_maps = _in_maps(in1, in2)
    res = run_bass_kernel_spmd(nc, in_maps, core_ids=list(range(B)))
    out = np.stack([res.results[b]["out"] for b in range(B)], axis=0)
    return out.astype(np.float32)


# revision 6
# speedup vs baseline: 1.0582x; 1.0582x over previous
"""Trainium2 Bass kernel for local cost-volume correlation (FlowNet-style), v3.

Problem: in1, in2 [B=8, C=256, H=96, W=128] fp32; out [B, 81, H, W] where
out[b, dy*9+dx, h, w] = mean_c in1[b,c,h,w] * in2[b,c,h+dy-4,w+dx-4] (zero pad).

Sharding: data-parallel over batch, one image per NeuronCore (8 cores).

v3 design (vs v2): R=8 row-packing, bias+max select, batched DMAs.
  - Host: in1, in2 scaled by 1/16 each (mean folded in) and cast to f16.
    Output f16, upcast on host.
  - Weight tile per (k, g): 8 in1 rows x 16 pixels (p = delta*16 + w2);
    rhs streams 16 in2 ring rows x 24-u window (N=384).  Each in2 row is
    streamed twice per k instead of 9x: PE cols/row = 2*384/8*8 = 768+tr.
  - in2 ring [P, 43, CK, 140] f16: slot(r) = (r+4) % 28, slots 0..14
    mirrored at +28 so any 16-slot window is contiguous.  One DMA per
    4-row quad (+1 mirror DMA), one iteration (8 rows) ahead.
  - Extraction per iter: psum[p=(delta,w2), j*24+u] --ACT--> S f16
    -> gather1 (3x8 chunks at g*48+(delta+dy)*3, delta = q//2... q=delta)
    -> mod-16 -> stream_shuffle (w2_3 <-> delta_0) -> gather2 (2 chunks,
    base + q%2) -> mod-8 -> ONE bias-add (0 at e==p%8 else -60000) +
    ONE max-reduce over the 8-window -> O[p, (g,dy,dx)].
  - 8 PE transposes (identity = inverse-shuffle perm), lagged one
    iteration; one ACT staging copy; one out-DMA per iter.
"""

import threading

import numpy as np

B, C, H, W = 8, 256, 96, 128
ND = 9             # displacement range per axis
NCH = ND * ND      # 81 output channels
CK = 2             # C // 128 contraction chunks
P = 128
RPI = 8            # rows per iteration
NT = 8             # pixel-group tiles per iteration (16 pixels each)
G = 16             # pixels per tile
JW = 16            # in2 row window per iteration (h-4 .. h+11)
UW = 24            # per-tile u window
BANDT = JW * UW    # 384, gram band per tile
RD = NT * ND       # 72 (g, dy) pairs per iteration
NSLOTP = 28        # physical ring slots
NMIR = 12          # slots 0..11 mirrored at +28
NSLOT = NSLOTP + NMIR  # 40
SROW = 140         # padded in2 row width (4 + 128 + 8)
G1D = 8            # gather chunk width
G1N = 112          # gather1 num_idxs per half-iteration (108 real + pad)
G2N = 160          # gather2 num_idxs (144 real + pad; pad also gives the
                   # +1-shifted q8b read one element of slack)
SCALE = 0.0625     # host-side per-input scale; SCALE^2 = 1/C

# stream_shuffle mask: swap bit3 (w2_3) with bit4 (delta_0) within each
# 32-partition quadrant
SHUF = list(range(0, 8)) + list(range(16, 24)) + list(range(8, 16)) + list(range(24, 32))

_cache = {}
_lock = threading.Lock()


def _wrap_idx(flat, ncols):
    n = flat.shape[1]
    out = np.zeros((P, ncols), dtype=np.int16)
    for q in range(8):
        for i in range(n):
            out[16 * q + (i % 16), i // 16] = flat[q, i]
    return out


def _host_tables():
    # gather1: S viewed [128, 384, 8]; for (g, dy) gather the whole 24-elem
    # u-window: 3 chunks at g*48 + (delta+dy)*3; delta = q.  -> mod-16.
    flat1 = np.zeros((8, G1N), dtype=np.int16)
    for q in range(8):
        delta = q
        for i in range(G1N):
            g, rem = divmod(min(i, 107), 27)
            dy, c = divmod(rem, 3)
            flat1[q, i] = g * 48 + (delta + dy) * 3 + c
    g1 = _wrap_idx(flat1, 7)

    # gather2: qs viewed [128, 216, 8]; for (g, dy) gather 2 chunks at
    # (g*9+dy)*3 + q%2 (q%2 = w2_3 after the shuffle).  -> mod-8.
    flat2 = np.zeros((8, G2N), dtype=np.int16)
    for q in range(8):
        for i in range(RD * 2):
            gd, c = divmod(i, 2)
            flat2[q, i] = gd * 3 + (q % 2) + c
    g2 = _wrap_idx(flat2, G2N // 16)

    # fully-materialized one-hot masks (no stride-0 dims -> DVE 2x mode)
    me = np.zeros((P, RD, 5, 8), dtype=np.float16)
    mo = np.zeros((P, RD, 4, 8), dtype=np.float16)
    for p in range(P):
        me[p, :, :, p % 8] = 1.0
        mo[p, :, :, p % 8] = 1.0
    # inverse shuffle permutation matrix: perm[p_new, old(p_new)] = 1
    perm = np.zeros((P, P), dtype=np.float16)
    for s in range(4):
        for i in range(32):
            perm[32 * s + i, 32 * s + SHUF[i]] = 1.0
    return g1, g2, me.reshape(P, -1), mo.reshape(P, -1), perm


def _build_nc():
    from contextlib import ExitStack

    import concourse.bass as bass
    import concourse.mybir as mybir
    import concourse.tile as tile
    from concourse import bacc

    f32 = mybir.dt.float32
    f16 = mybir.dt.float16
    i16 = mybir.dt.int16
    u32 = mybir.dt.uint32

    nc = bacc.Bacc("TRN2", target_bir_lowering=False, debug=False)
    # in1 pre-arranged on host to [p, hb, k, g, delta*16+w2] so each weight
    # tile (k, g) is a contiguous 128-column slab (walrus requires weight
    # APs to have a single free dimension).
    in1 = nc.declare_dram_parameter(
        "in1", [P, H // RPI, CK, NT, P], f16, isOutput=False
    )
    in2 = nc.declare_dram_parameter("in2", [C, H, W], f16, isOutput=False)
    g1t = nc.declare_dram_parameter("g1t", [P, 7], i16, isOutput=False)
    g2t = nc.declare_dram_parameter("g2t", [P, G2N // 16], i16, isOutput=False)
    met = nc.declare_dram_parameter("met", [P, RD * 40], f16, isOutput=False)
    mot = nc.declare_dram_parameter("mot", [P, RD * 32], f16, isOutput=False)
    permt = nc.declare_dram_parameter("permt", [P, P], f16, isOutput=False)
    out_t = nc.declare_dram_parameter("out", [NCH, H, W], f16, isOutput=True)

    # in2 accessed (h, k, w) to match the ring's [slot, k, u] layout
    in2r = in2[:].rearrange("(k p) h w -> p h k w", p=P)

    with ExitStack() as ctx:
        tc = ctx.enter_context(tile.TileContext(nc))
        const = ctx.enter_context(tc.tile_pool(name="const", bufs=1))
        persist = ctx.enter_context(tc.tile_pool(name="persist", bufs=1))
        inp = ctx.enter_context(tc.tile_pool(name="inp", bufs=3))
        sp = ctx.enter_context(tc.tile_pool(name="sp", bufs=3))
        qp = ctx.enter_context(tc.tile_pool(name="qp", bufs=3))
        qsp = ctx.enter_context(tc.tile_pool(name="qsp", bufs=3))
        q8p = ctx.enter_context(tc.tile_pool(name="q8p", bufs=4))
        pmp = ctx.enter_context(tc.tile_pool(name="pmp", bufs=2))
        op = ctx.enter_context(tc.tile_pool(name="op", bufs=4))
        stg = ctx.enter_context(tc.tile_pool(name="stg", bufs=3))
        gram = ctx.enter_context(tc.tile_pool(name="gram", bufs=6, space="PSUM"))
        ptp = ctx.enter_context(tc.tile_pool(name="ptp", bufs=2, space="PSUM"))

        g1_s = const.tile([P, 7], i16)
        nc.sync.dma_start(out=g1_s[:], in_=g1t[:])
        g2_s = const.tile([P, G2N // 16], i16)
        nc.sync.dma_start(out=g2_s[:], in_=g2t[:])
        me_s = const.tile([P, RD * 40], f16)
        nc.sync.dma_start(out=me_s[:], in_=met[:])
        mo_s = const.tile([P, RD * 32], f16)
        nc.sync.dma_start(out=mo_s[:], in_=mot[:])
        perm_s = const.tile([P, P], f16)
        nc.sync.dma_start(out=perm_s[:], in_=permt[:])

        # in2 ring: [p, slot, k, u]; slot(r) = (r+4) % 28, mirrored at +28
        # for slots 0..14 so the 16-slot j-window is always contiguous.
        Rr = persist.tile([P, NSLOT, CK, SROW], f16)
        # zero only what DMA never writes: the pad columns and the
        # initial rows<0 slots (0..3 and mirrors)
        nc.gpsimd.memset(Rr[:, :, :, 0:4], 0.0)
        nc.gpsimd.memset(Rr[:, :, :, 4 + W : SROW], 0.0)
        nc.gpsimd.memset(Rr[:, 0:4, :, 4 : 4 + W], 0.0)
        nc.gpsimd.memset(Rr[:, NSLOTP : NSLOTP + 4, :, 4 : 4 + W], 0.0)

        def ring_wr(s, k, r, n):
            Rra = Rr[:]
            dst = bass.AP(
                tensor=Rra.tensor,
                offset=Rra.offset + s * (CK * SROW) + k * SROW + 4,
                ap=[Rra.ap[0], [CK * SROW, n], [1, W]],
            )
            nc.sync.dma_start(out=dst, in_=in2r[:, r : r + n, k, :])

        def load_in2_8rows(r):
            s = (r + 4) % NSLOTP
            n1 = min(8, NSLOTP - s)  # rows before wrapping past slot 27
            for k in range(CK):
                ring_wr(s, k, r, n1)
                if n1 < 8:
                    ring_wr(0, k, r + n1, 8 - n1)
                # mirror rows landing at slots 0..NMIR-1
                if s < NMIR:
                    ring_wr(s + NSLOTP, k, r, min(n1, NMIR - s))
                if n1 < 8:
                    ring_wr(NSLOTP, k, r + n1, 8 - n1)

        def zero_in2_quads(r):
            # rows r..r+7 are past the image: zero their slots (never in
            # the mirror range for our h grid)
            s = (r + 4) % NSLOTP
            nc.gpsimd.memset(Rr[:, s : s + 8, :, 4 : 4 + W], 0.0)

        win_tiles = {}

        def load_win(hh):
            t = inp.tile([P, CK, NT, P], f16, tag="win")
            nc.scalar.dma_start(out=t[:], in_=in1[:][:, hh // RPI])
            win_tiles[hh] = t

        load_in2_8rows(0)
        load_in2_8rows(8)
        load_win(0)
        load_win(8)

        def emit_output(O, h0):
            # 8 transposes into one psum tile; identity = inverse shuffle
            # permutation so columns land at true (delta, w2).
            pt = ptp.tile([P, NT, P], f16, tag="pt")
            for g in range(NT):
                nc.tensor.transpose(
                    out=pt[0:NCH, g, :],
                    in_=O[:, g * NCH : (g + 1) * NCH],
                    identity=perm_s[:],
                )
            Ost = stg.tile([NCH, RPI, W], f16)
            pta = pt[0:NCH]
            # pt[ch, g, (delta, w2)] -> Ost[ch, delta, g*16 + w2]
            src = bass.AP(
                tensor=pta.tensor,
                offset=pta.offset,
                ap=[pta.ap[0], [G, RPI], [P, NT], [1, G]],
            )
            Oa = Ost[:]
            dst = bass.AP(
                tensor=Oa.tensor,
                offset=Oa.offset,
                ap=[Oa.ap[0], [W, RPI], [G, NT], [1, G]],
            )
            nc.scalar.copy(out=dst, in_=src)
            nc.scalar.dma_start(out=out_t[:, h0 : h0 + RPI, :], in_=Ost[:])

        def run_tree(q8):
            # masked select: value for (p, rd, dxi) at q8[p, rd*16 + dxi + p%8].
            # Even dxi read q8 directly (4B-aligned); odd dxi read a +1-shifted
            # copy so their windows start even too -> everything runs 2x.
            q8a = q8[:].rearrange("p a b -> p (a b)")
            q8b = q8p.tile([P, RD * 16], f16, name="q8b", tag="q8b")
            shift_src = bass.AP(
                tensor=q8a.tensor, offset=q8a.offset + 1,
                ap=[q8a.ap[0], [1, RD * 16]],
            )
            nc.scalar.copy(out=q8b[:], in_=shift_src)

            in_e = bass.AP(
                tensor=q8a.tensor, offset=q8a.offset,
                ap=[q8a.ap[0], [16, RD], [2, 5], [1, 8]],
            )
            q8ba = q8b[:]
            in_o = bass.AP(
                tensor=q8ba.tensor, offset=q8ba.offset,
                ap=[q8ba.ap[0], [16, RD], [2, 4], [1, 8]],
            )
            pe_ = pmp.tile([P, RD, 5, 8], f16, name="pe", tag="pe")
            po_ = pmp.tile([P, RD, 4, 8], f16, name="po", tag="po")
            nc.vector.tensor_mul(
                pe_[:], in_e,
                me_s[:].rearrange("p (a b c) -> p a b c", b=5, c=8),
            )
            nc.vector.tensor_mul(
                po_[:], in_o,
                mo_s[:].rearrange("p (a b c) -> p a b c", b=4, c=8),
            )

            t1e = op.tile([P, RD, 5, 4], f16, name="t1e", tag="t1e")
            t1o = op.tile([P, RD, 4, 4], f16, name="t1o", tag="t1o")
            nc.vector.tensor_add(t1e[:], pe_[:, :, :, 0:4], pe_[:, :, :, 4:8])
            nc.vector.tensor_add(t1o[:], po_[:, :, :, 0:4], po_[:, :, :, 4:8])
            t2e = op.tile([P, RD, 5, 2], f16, name="t2e", tag="t2e")
            t2o = op.tile([P, RD, 4, 2], f16, name="t2o", tag="t2o")
            nc.vector.tensor_add(t2e[:], t1e[:, :, :, 0:2], t1e[:, :, :, 2:4])
            nc.vector.tensor_add(t2o[:], t1o[:, :, :, 0:2], t1o[:, :, :, 2:4])

            O = op.tile([P, RD * ND], f16, name="O", tag="O")
            Oa = O[:]
            out_e = bass.AP(
                tensor=Oa.tensor, offset=Oa.offset,
                ap=[Oa.ap[0], [ND, RD], [2, 5]],
            )
            out_o = bass.AP(
                tensor=Oa.tensor, offset=Oa.offset + 1,
                ap=[Oa.ap[0], [ND, RD], [2, 4]],
            )
            nc.vector.tensor_add(out_e, t2e[:, :, :, 0], t2e[:, :, :, 1])
            nc.vector.tensor_add(out_o, t2o[:, :, :, 0], t2o[:, :, :, 1])
            return O

        pend_tree = None  # (q8, h): select tree lagged one iteration
        pend_emit = []    # [(O, h)]: transposes/staging lagged two
        for h in range(0, H, RPI):
            # prefetch in2 rows h+16..h+23 (one iteration ahead)
            if h + 16 < H:
                load_in2_8rows(h + 16)
            elif h + 16 == H:
                zero_in2_quads(h + 16)
            if h + 16 < H:
                load_win(h + 16)

            win = win_tiles.pop(h)
            wa = win[:]

            # R=8 packed matmuls: per tile g, weights = 8 rows x 16 pixels
            # (p = delta*16 + w2), rhs = 16 ring rows x 24-u window, N=384.
            s0 = h % NSLOTP
            pss = []
            for g in range(NT):
                ps = gram.tile([P, BANDT], f32, tag="ps")
                for k in range(CK):
                    lhsT = bass.AP(
                        tensor=wa.tensor,
                        offset=wa.offset + (k * NT + g) * P,
                        ap=[wa.ap[0], [1, P]],
                    )
                    rhs_t = Rr[:]
                    rhs = bass.AP(
                        tensor=rhs_t.tensor,
                        offset=rhs_t.offset
                        + s0 * (CK * SROW)
                        + k * SROW
                        + G * g,
                        ap=[rhs_t.ap[0], [CK * SROW, JW], [1, UW]],
                    )
                    nc.tensor.matmul(
                        out=ps[:],
                        lhsT=lhsT,
                        rhs=rhs,
                        start=(k == 0),
                        stop=(k == CK - 1),
                    )
                pss.append(ps)

            S = sp.tile([P, NT, BANDT], f16)
            for g in range(NT):
                nc.scalar.copy(out=S[:, g, :], in_=pss[g][:])

            # gather1 -> mod-16 (whole 24-elem windows; j = delta+dy select).
            # Two half-gathers (4 tiles each, same table) so the first can
            # start as soon as the first four band copies land; data moved
            # as uint32 pairs: same 16-byte chunks, half the gpsimd cost.
            q16 = qp.tile([P, 2 * G1N, G1D], f16)
            q16f = q16[:].rearrange("p a b -> p (a b)")
            for half in range(2):
                nc.gpsimd.ap_gather(
                    out_ap=q16f[:, half * G1N * G1D : (half + 1) * G1N * G1D]
                    .bitcast(u32)
                    .rearrange("p (n d) -> p n d", d=G1D // 2),
                    in_ap=S[:, 4 * half : 4 * half + 4, :]
                    .rearrange("p a b -> p (a b)")
                    .bitcast(u32)
                    .rearrange("p (n d) -> p n d", d=G1D // 2),
                    idxs_ap=g1_s[:],
                    channels=P,
                    num_elems=NT * BANDT // (2 * G1D),
                    d=G1D // 2,
                    num_idxs=G1N,
                )

            # run the PREVIOUS iteration's select tree first: its inputs are
            # ready, so DVE stays busy while Pool finishes gather1 above.
            if pend_tree is not None:
                q8_p, h_p = pend_tree
                pend_emit.append((run_tree(q8_p), h_p))
                pend_tree = None

            qs = qsp.tile([P, RD * 24], f16)
            for half in range(2):
                nc.vector.stream_shuffle(
                    out=qs[:, half * 864 : (half + 1) * 864].bitcast(u32),
                    in_=q16f[:, half * G1N * G1D : half * G1N * G1D + 864].bitcast(
                        u32
                    ),
                    mask=SHUF,
                )

            # gather2 -> mod-8 (uint32 pairs again)
            q8 = q8p.tile([P, G2N, G1D], f16)
            nc.gpsimd.ap_gather(
                out_ap=q8[:].rearrange("p a b -> p (a b)").bitcast(u32).rearrange(
                    "p (n d) -> p n d", d=G1D // 2
                ),
                in_ap=qs[:].bitcast(u32).rearrange("p (n d) -> p n d", d=G1D // 2),
                idxs_ap=g2_s[:],
                channels=P,
                num_elems=RD * 24 // G1D,
                d=G1D // 2,
                num_idxs=G2N,
            )

            pend_tree = (q8, h)

            if len(pend_emit) >= 2:
                emit_output(*pend_emit.pop(0))

        if pend_tree is not None:
            q8_p, h_p = pend_tree
            pend_emit.append((run_tree(q8_p), h_p))
        for item in pend_emit:
            emit_output(*item)

    nc.finalize()
    return nc


def _get_nc():
    with _lock:
        if "nc" not in _cache:
            _cache["nc"] = _build_nc()
        return _cache["nc"]


def _in_maps(in1: np.ndarray, in2: np.ndarray):
    g1, g2, me, mo, perm = _host_tables()
    in1 = (np.ascontiguousarray(in1) * SCALE).astype(np.float16)
    in2 = (np.ascontiguousarray(in2) * SCALE).astype(np.float16)
    # [b, c=(k p), h=(hb delta), w=(g w2)] -> [b, p, hb, k, g, (delta w2)]
    in1 = np.ascontiguousarray(
        in1.reshape(B, CK, P, H // RPI, RPI, NT, G).transpose(0, 2, 3, 1, 5, 4, 6)
    ).reshape(B, P, H // RPI, CK, NT, P)
    return [
        {
            "in1": in1[b],
            "in2": in2[b],
            "g1t": g1,
            "g2t": g2,
            "met": me,
            "mot": mo,
            "permt": perm,
        }
        for b in range(B)
    ]


def kernel(in1: np.ndarray, in2: np.ndarray) -> np.ndarray:
    from concourse.bass_utils import run_bass_kernel_spmd

    nc = _get_nc()
    in_maps = _in_maps(in1, in2)
    res = run_bass_kernel_spmd(nc, in_maps, core_ids=list(range(B)))
    out = np.stack([res.results[b]["out"] for b in range(B)], axis=0)
    return out.astype(np.float32)


# revision 7
# speedup vs baseline: 1.0610x; 1.0027x over previous
"""Trainium2 Bass kernel for local cost-volume correlation (FlowNet-style), v3.

Problem: in1, in2 [B=8, C=256, H=96, W=128] fp32; out [B, 81, H, W] where
out[b, dy*9+dx, h, w] = mean_c in1[b,c,h,w] * in2[b,c,h+dy-4,w+dx-4] (zero pad).

Sharding: data-parallel over batch, one image per NeuronCore (8 cores).

v3 design (vs v2): R=8 row-packing, bias+max select, batched DMAs.
  - Host: in1, in2 scaled by 1/16 each (mean folded in) and cast to f16.
    Output f16, upcast on host.
  - Weight tile per (k, g): 8 in1 rows x 16 pixels (p = delta*16 + w2);
    rhs streams 16 in2 ring rows x 24-u window (N=384).  Each in2 row is
    streamed twice per k instead of 9x: PE cols/row = 2*384/8*8 = 768+tr.
  - in2 ring [P, 43, CK, 140] f16: slot(r) = (r+4) % 28, slots 0..14
    mirrored at +28 so any 16-slot window is contiguous.  One DMA per
    4-row quad (+1 mirror DMA), one iteration (8 rows) ahead.
  - Extraction per iter: psum[p=(delta,w2), j*24+u] --ACT--> S f16
    -> gather1 (3x8 chunks at g*48+(delta+dy)*3, delta = q//2... q=delta)
    -> mod-16 -> stream_shuffle (w2_3 <-> delta_0) -> gather2 (2 chunks,
    base + q%2) -> mod-8 -> ONE bias-add (0 at e==p%8 else -60000) +
    ONE max-reduce over the 8-window -> O[p, (g,dy,dx)].
  - 8 PE transposes (identity = inverse-shuffle perm), lagged one
    iteration; one ACT staging copy; one out-DMA per iter.
"""

import threading

import numpy as np

B, C, H, W = 8, 256, 96, 128
ND = 9             # displacement range per axis
NCH = ND * ND      # 81 output channels
CK = 2             # C // 128 contraction chunks
P = 128
RPI = 8            # rows per iteration
NT = 8             # pixel-group tiles per iteration (16 pixels each)
G = 16             # pixels per tile
JW = 16            # in2 row window per iteration (h-4 .. h+11)
UW = 24            # per-tile u window
BANDT = JW * UW    # 384, gram band per tile
RD = NT * ND       # 72 (g, dy) pairs per iteration
NSLOTP = 28        # physical ring slots
NMIR = 12          # slots 0..11 mirrored at +28
NSLOT = NSLOTP + NMIR  # 40
SROW = 140         # padded in2 row width (4 + 128 + 8)
G1D = 8            # gather chunk width
G1N = 112          # gather1 num_idxs per half-iteration (108 real + pad)
G2N = 160          # gather2 num_idxs (144 real + pad; pad also gives the
                   # +1-shifted q8b read one element of slack)
SCALE = 0.0625     # host-side per-input scale; SCALE^2 = 1/C

# stream_shuffle mask: swap bit3 (w2_3) with bit4 (delta_0) within each
# 32-partition quadrant
SHUF = list(range(0, 8)) + list(range(16, 24)) + list(range(8, 16)) + list(range(24, 32))

_cache = {}
_lock = threading.Lock()


def _wrap_idx(flat, ncols):
    n = flat.shape[1]
    out = np.zeros((P, ncols), dtype=np.int16)
    for q in range(8):
        for i in range(n):
            out[16 * q + (i % 16), i // 16] = flat[q, i]
    return out


def _host_tables():
    # gather1: S viewed [128, 384, 8]; for (g, dy) gather the whole 24-elem
    # u-window: 3 chunks at g*48 + (delta+dy)*3; delta = q.  -> mod-16.
    flat1 = np.zeros((8, G1N), dtype=np.int16)
    for q in range(8):
        delta = q
        for i in range(G1N):
            g, rem = divmod(min(i, 107), 27)
            dy, c = divmod(rem, 3)
            flat1[q, i] = g * 48 + (delta + dy) * 3 + c
    g1 = _wrap_idx(flat1, 7)

    # gather2: qs viewed [128, 216, 8]; for (g, dy) gather 2 chunks at
    # (g*9+dy)*3 + q%2 (q%2 = w2_3 after the shuffle).  -> mod-8.
    flat2 = np.zeros((8, G2N), dtype=np.int16)
    for q in range(8):
        for i in range(RD * 2):
            gd, c = divmod(i, 2)
            flat2[q, i] = gd * 3 + (q % 2) + c
    g2 = _wrap_idx(flat2, G2N // 16)

    # fully-materialized one-hot masks (no stride-0 dims -> DVE 2x mode)
    me = np.zeros((P, RD, 5, 8), dtype=np.float16)
    mo = np.zeros((P, RD, 4, 8), dtype=np.float16)
    for p in range(P):
        me[p, :, :, p % 8] = 1.0
        mo[p, :, :, p % 8] = 1.0
    # inverse shuffle permutation matrix: perm[p_new, old(p_new)] = 1
    perm = np.zeros((P, P), dtype=np.float16)
    for s in range(4):
        for i in range(32):
            perm[32 * s + i, 32 * s + SHUF[i]] = 1.0
    return g1, g2, me.reshape(P, -1), mo.reshape(P, -1), perm


def _build_nc():
    from contextlib import ExitStack

    import concourse.bass as bass
    import concourse.mybir as mybir
    import concourse.tile as tile
    from concourse import bacc

    f32 = mybir.dt.float32
    f16 = mybir.dt.float16
    i16 = mybir.dt.int16
    u32 = mybir.dt.uint32

    nc = bacc.Bacc("TRN2", target_bir_lowering=False, debug=False)
    # in1 pre-arranged on host to [p, hb, k, g, delta*16+w2] so each weight
    # tile (k, g) is a contiguous 128-column slab (walrus requires weight
    # APs to have a single free dimension).
    in1 = nc.declare_dram_parameter(
        "in1", [P, H // RPI, CK, NT, P], f16, isOutput=False
    )
    in2 = nc.declare_dram_parameter("in2", [C, H, W], f16, isOutput=False)
    g1t = nc.declare_dram_parameter("g1t", [P, 7], i16, isOutput=False)
    g2t = nc.declare_dram_parameter("g2t", [P, G2N // 16], i16, isOutput=False)
    met = nc.declare_dram_parameter("met", [P, RD * 40], f16, isOutput=False)
    mot = nc.declare_dram_parameter("mot", [P, RD * 32], f16, isOutput=False)
    permt = nc.declare_dram_parameter("permt", [P, P], f16, isOutput=False)
    out_t = nc.declare_dram_parameter("out", [NCH, H, W], f16, isOutput=True)

    # in2 accessed (h, k, w) to match the ring's [slot, k, u] layout
    in2r = in2[:].rearrange("(k p) h w -> p h k w", p=P)

    with ExitStack() as ctx:
        tc = ctx.enter_context(tile.TileContext(nc))
        const = ctx.enter_context(tc.tile_pool(name="const", bufs=1))
        persist = ctx.enter_context(tc.tile_pool(name="persist", bufs=1))
        inp = ctx.enter_context(tc.tile_pool(name="inp", bufs=3))
        sp = ctx.enter_context(tc.tile_pool(name="sp", bufs=3))
        qp = ctx.enter_context(tc.tile_pool(name="qp", bufs=3))
        qsp = ctx.enter_context(tc.tile_pool(name="qsp", bufs=3))
        q8p = ctx.enter_context(tc.tile_pool(name="q8p", bufs=4))
        pmp = ctx.enter_context(tc.tile_pool(name="pmp", bufs=2))
        op = ctx.enter_context(tc.tile_pool(name="op", bufs=4))
        stg = ctx.enter_context(tc.tile_pool(name="stg", bufs=3))
        gram = ctx.enter_context(tc.tile_pool(name="gram", bufs=7, space="PSUM"))
        ptp = ctx.enter_context(tc.tile_pool(name="ptp", bufs=1, space="PSUM"))

        g1_s = const.tile([P, 7], i16)
        nc.sync.dma_start(out=g1_s[:], in_=g1t[:])
        g2_s = const.tile([P, G2N // 16], i16)
        nc.sync.dma_start(out=g2_s[:], in_=g2t[:])
        me_s = const.tile([P, RD * 40], f16)
        nc.sync.dma_start(out=me_s[:], in_=met[:])
        mo_s = const.tile([P, RD * 32], f16)
        nc.sync.dma_start(out=mo_s[:], in_=mot[:])
        perm_s = const.tile([P, P], f16)
        nc.sync.dma_start(out=perm_s[:], in_=permt[:])

        # in2 ring: [p, slot, k, u]; slot(r) = (r+4) % 28, mirrored at +28
        # for slots 0..14 so the 16-slot j-window is always contiguous.
        Rr = persist.tile([P, NSLOT, CK, SROW], f16)
        # zero only what DMA never writes: the pad columns and the
        # initial rows<0 slots (0..3 and mirrors)
        nc.gpsimd.memset(Rr[:, :, :, 0:4], 0.0)
        nc.gpsimd.memset(Rr[:, :, :, 4 + W : SROW], 0.0)
        nc.gpsimd.memset(Rr[:, 0:4, :, 4 : 4 + W], 0.0)
        nc.gpsimd.memset(Rr[:, NSLOTP : NSLOTP + 4, :, 4 : 4 + W], 0.0)

        def ring_wr(s, k, r, n):
            Rra = Rr[:]
            dst = bass.AP(
                tensor=Rra.tensor,
                offset=Rra.offset + s * (CK * SROW) + k * SROW + 4,
                ap=[Rra.ap[0], [CK * SROW, n], [1, W]],
            )
            nc.sync.dma_start(out=dst, in_=in2r[:, r : r + n, k, :])

        def load_in2_8rows(r):
            s = (r + 4) % NSLOTP
            n1 = min(8, NSLOTP - s)  # rows before wrapping past slot 27
            for k in range(CK):
                ring_wr(s, k, r, n1)
                if n1 < 8:
                    ring_wr(0, k, r + n1, 8 - n1)
                # mirror rows landing at slots 0..NMIR-1
                if s < NMIR:
                    ring_wr(s + NSLOTP, k, r, min(n1, NMIR - s))
                if n1 < 8:
                    ring_wr(NSLOTP, k, r + n1, 8 - n1)

        def zero_in2_quads(r):
            # rows r..r+7 are past the image: zero their slots (never in
            # the mirror range for our h grid)
            s = (r + 4) % NSLOTP
            nc.gpsimd.memset(Rr[:, s : s + 8, :, 4 : 4 + W], 0.0)

        win_tiles = {}

        def load_win(hh):
            t = inp.tile([P, CK, NT, P], f16, tag="win")
            nc.scalar.dma_start(out=t[:], in_=in1[:][:, hh // RPI])
            win_tiles[hh] = t

        load_in2_8rows(0)
        load_in2_8rows(8)
        load_win(0)
        load_win(8)

        def emit_output(O, h0):
            # 8 transposes into one psum tile; identity = inverse shuffle
            # permutation so columns land at true (delta, w2).
            pt = ptp.tile([P, NT, P], f16, tag="pt")
            for g in range(NT):
                nc.tensor.transpose(
                    out=pt[0:NCH, g, :],
                    in_=O[:, g * NCH : (g + 1) * NCH],
                    identity=perm_s[:],
                )
            Ost = stg.tile([NCH, RPI, W], f16)
            pta = pt[0:NCH]
            # pt[ch, g, (delta, w2)] -> Ost[ch, delta, g*16 + w2]
            src = bass.AP(
                tensor=pta.tensor,
                offset=pta.offset,
                ap=[pta.ap[0], [G, RPI], [P, NT], [1, G]],
            )
            Oa = Ost[:]
            dst = bass.AP(
                tensor=Oa.tensor,
                offset=Oa.offset,
                ap=[Oa.ap[0], [W, RPI], [G, NT], [1, G]],
            )
            nc.scalar.copy(out=dst, in_=src)
            nc.scalar.dma_start(out=out_t[:, h0 : h0 + RPI, :], in_=Ost[:])

        def run_tree(q8):
            # masked select: value for (p, rd, dxi) at q8[p, rd*16 + dxi + p%8].
            # Even dxi read q8 directly (4B-aligned); odd dxi read a +1-shifted
            # copy so their windows start even too -> everything runs 2x.
            q8a = q8[:].rearrange("p a b -> p (a b)")
            q8b = q8p.tile([P, RD * 16], f16, name="q8b", tag="q8b")
            shift_src = bass.AP(
                tensor=q8a.tensor, offset=q8a.offset + 1,
                ap=[q8a.ap[0], [1, RD * 16]],
            )
            nc.scalar.copy(out=q8b[:], in_=shift_src)

            in_e = bass.AP(
                tensor=q8a.tensor, offset=q8a.offset,
                ap=[q8a.ap[0], [16, RD], [2, 5], [1, 8]],
            )
            q8ba = q8b[:]
            in_o = bass.AP(
                tensor=q8ba.tensor, offset=q8ba.offset,
                ap=[q8ba.ap[0], [16, RD], [2, 4], [1, 8]],
            )
            pe_ = pmp.tile([P, RD, 5, 8], f16, name="pe", tag="pe")
            po_ = pmp.tile([P, RD, 4, 8], f16, name="po", tag="po")
            nc.vector.tensor_mul(
                pe_[:], in_e,
                me_s[:].rearrange("p (a b c) -> p a b c", b=5, c=8),
            )
            nc.vector.tensor_mul(
                po_[:], in_o,
                mo_s[:].rearrange("p (a b c) -> p a b c", b=4, c=8),
            )

            t1e = op.tile([P, RD, 5, 4], f16, name="t1e", tag="t1e")
            t1o = op.tile([P, RD, 4, 4], f16, name="t1o", tag="t1o")
            nc.vector.tensor_add(t1e[:], pe_[:, :, :, 0:4], pe_[:, :, :, 4:8])
            nc.vector.tensor_add(t1o[:], po_[:, :, :, 0:4], po_[:, :, :, 4:8])
            t2e = op.tile([P, RD, 5, 2], f16, name="t2e", tag="t2e")
            t2o = op.tile([P, RD, 4, 2], f16, name="t2o", tag="t2o")
            nc.vector.tensor_add(t2e[:], t1e[:, :, :, 0:2], t1e[:, :, :, 2:4])
            nc.vector.tensor_add(t2o[:], t1o[:, :, :, 0:2], t1o[:, :, :, 2:4])

            O = op.tile([P, RD * ND], f16, name="O", tag="O")
            Oa = O[:]
            out_e = bass.AP(
                tensor=Oa.tensor, offset=Oa.offset,
                ap=[Oa.ap[0], [ND, RD], [2, 5]],
            )
            out_o = bass.AP(
                tensor=Oa.tensor, offset=Oa.offset + 1,
                ap=[Oa.ap[0], [ND, RD], [2, 4]],
            )
            nc.vector.tensor_add(out_e, t2e[:, :, :, 0], t2e[:, :, :, 1])
            nc.vector.tensor_add(out_o, t2o[:, :, :, 0], t2o[:, :, :, 1])
            return O

        pend_tree = None  # (q8, h): select tree lagged one iteration
        pend_emit = []    # [(O, h)]: transposes/staging lagged two
        for h in range(0, H, RPI):
            # prefetch in2 rows h+16..h+23 (one iteration ahead)
            if h + 16 < H:
                load_in2_8rows(h + 16)
            elif h + 16 == H:
                zero_in2_quads(h + 16)
            if h + 16 < H:
                load_win(h + 16)

            win = win_tiles.pop(h)
            wa = win[:]

            # R=8 packed matmuls: per tile g, weights = 8 rows x 16 pixels
            # (p = delta*16 + w2), rhs = 16 ring rows x 24-u window, N=384.
            s0 = h % NSLOTP
            pss = []
            for g in range(NT):
                ps = gram.tile([P, BANDT], f32, tag="ps")
                for k in range(CK):
                    lhsT = bass.AP(
                        tensor=wa.tensor,
                        offset=wa.offset + (k * NT + g) * P,
                        ap=[wa.ap[0], [1, P]],
                    )
                    rhs_t = Rr[:]
                    rhs = bass.AP(
                        tensor=rhs_t.tensor,
                        offset=rhs_t.offset
                        + s0 * (CK * SROW)
                        + k * SROW
                        + G * g,
                        ap=[rhs_t.ap[0], [CK * SROW, JW], [1, UW]],
                    )
                    nc.tensor.matmul(
                        out=ps[:],
                        lhsT=lhsT,
                        rhs=rhs,
                        start=(k == 0),
                        stop=(k == CK - 1),
                    )
                pss.append(ps)

            S = sp.tile([P, NT, BANDT], f16)
            for g in range(NT):
                nc.scalar.copy(out=S[:, g, :], in_=pss[g][:])

            # gather1 -> mod-16 (whole 24-elem windows; j = delta+dy select).
            # Two half-gathers (4 tiles each, same table) so the first can
            # start as soon as the first four band copies land; data moved
            # as uint32 pairs: same 16-byte chunks, half the gpsimd cost.
            q16 = qp.tile([P, 2 * G1N, G1D], f16)
            q16f = q16[:].rearrange("p a b -> p (a b)")
            for half in range(2):
                nc.gpsimd.ap_gather(
                    out_ap=q16f[:, half * G1N * G1D : (half + 1) * G1N * G1D]
                    .bitcast(u32)
                    .rearrange("p (n d) -> p n d", d=G1D // 2),
                    in_ap=S[:, 4 * half : 4 * half + 4, :]
                    .rearrange("p a b -> p (a b)")
                    .bitcast(u32)
                    .rearrange("p (n d) -> p n d", d=G1D // 2),
                    idxs_ap=g1_s[:],
                    channels=P,
                    num_elems=NT * BANDT // (2 * G1D),
                    d=G1D // 2,
                    num_idxs=G1N,
                )

            # run the PREVIOUS iteration's select tree first: its inputs are
            # ready, so DVE stays busy while Pool finishes gather1 above.
            if pend_tree is not None:
                q8_p, h_p = pend_tree
                pend_emit.append((run_tree(q8_p), h_p))
                pend_tree = None

            qs = qsp.tile([P, RD * 24], f16)
            for half in range(2):
                nc.vector.stream_shuffle(
                    out=qs[:, half * 864 : (half + 1) * 864].bitcast(u32),
                    in_=q16f[:, half * G1N * G1D : half * G1N * G1D + 864].bitcast(
                        u32
                    ),
                    mask=SHUF,
                )

            # gather2 -> mod-8 (uint32 pairs again)
            q8 = q8p.tile([P, G2N, G1D], f16)
            nc.gpsimd.ap_gather(
                out_ap=q8[:].rearrange("p a b -> p (a b)").bitcast(u32).rearrange(
                    "p (n d) -> p n d", d=G1D // 2
                ),
                in_ap=qs[:].bitcast(u32).rearrange("p (n d) -> p n d", d=G1D // 2),
                idxs_ap=g2_s[:],
                channels=P,
                num_elems=RD * 24 // G1D,
                d=G1D // 2,
                num_idxs=G2N,
            )

            pend_tree = (q8, h)

            if len(pend_emit) >= 2:
                emit_output(*pend_emit.pop(0))

        if pend_tree is not None:
            q8_p, h_p = pend_tree
            pend_emit.append((run_tree(q8_p), h_p))
        for item in pend_emit:
            emit_output(*item)

    nc.finalize()
    return nc


def _get_nc():
    with _lock:
        if "nc" not in _cache:
            _cache["nc"] = _build_nc()
        return _cache["nc"]


def _in_maps(in1: np.ndarray, in2: np.ndarray):
    g1, g2, me, mo, perm = _host_tables()
    in1 = (np.ascontiguousarray(in1) * SCALE).astype(np.float16)
    in2 = (np.ascontiguousarray(in2) * SCALE).astype(np.float16)
    # [b, c=(k p), h=(hb delta), w=(g w2)] -> [b, p, hb, k, g, (delta w2)]
    in1 = np.ascontiguousarray(
        in1.reshape(B, CK, P, H // RPI, RPI, NT, G).transpose(0, 2, 3, 1, 5, 4, 6)
    ).reshape(B, P, H // RPI, CK, NT, P)
    return [
        {
            "in1": in1[b],
            "in2": in2[b],
            "g1t": g1,
            "g2t": g2,
            "met": me,
            "mot": mo,
            "permt": perm,
        }
        for b in range(B)
    ]


def kernel(in1: np.ndarray, in2: np.ndarray) -> np.ndarray:
    from concourse.bass_utils import run_bass_kernel_spmd

    nc = _get_nc()
    in_maps = _in_maps(in1, in2)
    res = run_bass_kernel_spmd(nc, in_maps, core_ids=list(range(B)))
    out = np.stack([res.results[b]["out"] for b in range(B)], axis=0)
    return out.astype(np.float32)


# revision 8
# speedup vs baseline: 1.0774x; 1.0154x over previous
"""Trainium2 Bass kernel for local cost-volume correlation (FlowNet-style), v3.

Problem: in1, in2 [B=8, C=256, H=96, W=128] fp32; out [B, 81, H, W] where
out[b, dy*9+dx, h, w] = mean_c in1[b,c,h,w] * in2[b,c,h+dy-4,w+dx-4] (zero pad).

Sharding: data-parallel over batch, one image per NeuronCore (8 cores).

v3 design (vs v2): R=8 row-packing, bias+max select, batched DMAs.
  - Host: in1, in2 scaled by 1/16 each (mean folded in) and cast to f16.
    Output f16, upcast on host.
  - Weight tile per (k, g): 8 in1 rows x 16 pixels (p = delta*16 + w2);
    rhs streams 16 in2 ring rows x 24-u window (N=384).  Each in2 row is
    streamed twice per k instead of 9x: PE cols/row = 2*384/8*8 = 768+tr.
  - in2 ring [P, 43, CK, 140] f16: slot(r) = (r+4) % 28, slots 0..14
    mirrored at +28 so any 16-slot window is contiguous.  One DMA per
    4-row quad (+1 mirror DMA), one iteration (8 rows) ahead.
  - Extraction per iter: psum[p=(delta,w2), j*24+u] --ACT--> S f16
    -> gather1 (3x8 chunks at g*48+(delta+dy)*3, delta = q//2... q=delta)
    -> mod-16 -> stream_shuffle (w2_3 <-> delta_0) -> gather2 (2 chunks,
    base + q%2) -> mod-8 -> ONE bias-add (0 at e==p%8 else -60000) +
    ONE max-reduce over the 8-window -> O[p, (g,dy,dx)].
  - 8 PE transposes (identity = inverse-shuffle perm), lagged one
    iteration; one ACT staging copy; one out-DMA per iter.
"""

import threading

import numpy as np

B, C, H, W = 8, 256, 96, 128
ND = 9             # displacement range per axis
NCH = ND * ND      # 81 output channels
CK = 2             # C // 128 contraction chunks
P = 128
RPI = 8            # rows per iteration
NT = 8             # pixel-group tiles per iteration (16 pixels each)
G = 16             # pixels per tile
JW = 16            # in2 row window per iteration (h-4 .. h+11)
UW = 24            # per-tile u window
BANDT = JW * UW    # 384, gram band per tile
RD = NT * ND       # 72 (g, dy) pairs per iteration
NSLOTP = 28        # physical ring slots
NMIR = 12          # slots 0..11 mirrored at +28
NSLOT = NSLOTP + NMIR  # 40
SROW = 140         # padded in2 row width (4 + 128 + 8)
G1D = 8            # gather chunk width
G1N = 112          # gather1 num_idxs per half-iteration (108 real + pad)
G2N = 160          # gather2 num_idxs (144 real + pad; pad also gives the
                   # +1-shifted q8b read one element of slack)
SCALE = 0.0625     # host-side per-input scale; SCALE^2 = 1/C

# stream_shuffle mask: swap bit3 (w2_3) with bit4 (delta_0) within each
# 32-partition quadrant
SHUF = list(range(0, 8)) + list(range(16, 24)) + list(range(8, 16)) + list(range(24, 32))

_cache = {}
_lock = threading.Lock()


def _wrap_idx(flat, ncols):
    n = flat.shape[1]
    out = np.zeros((P, ncols), dtype=np.int16)
    for q in range(8):
        for i in range(n):
            out[16 * q + (i % 16), i // 16] = flat[q, i]
    return out


def _host_tables():
    # gather1: S viewed [128, 384, 8]; for (g, dy) gather the whole 24-elem
    # u-window: 3 chunks at g*48 + (delta+dy)*3; delta = q.  -> mod-16.
    flat1 = np.zeros((8, G1N), dtype=np.int16)
    for q in range(8):
        delta = q
        for i in range(G1N):
            g, rem = divmod(min(i, 107), 27)
            dy, c = divmod(rem, 3)
            flat1[q, i] = g * 48 + (delta + dy) * 3 + c
    g1 = _wrap_idx(flat1, 7)

    # gather2: qs viewed [128, 216, 8]; for (g, dy) gather 2 chunks at
    # (g*9+dy)*3 + q%2 (q%2 = w2_3 after the shuffle).  -> mod-8.
    flat2 = np.zeros((8, G2N), dtype=np.int16)
    for q in range(8):
        for i in range(RD * 2):
            gd, c = divmod(i, 2)
            flat2[q, i] = gd * 3 + (q % 2) + c
    g2 = _wrap_idx(flat2, G2N // 16)

    # fully-materialized one-hot masks (no stride-0 dims -> DVE 2x mode)
    me = np.zeros((P, RD, 5, 8), dtype=np.float16)
    mo = np.zeros((P, RD, 4, 8), dtype=np.float16)
    for p in range(P):
        me[p, :, :, p % 8] = 1.0
        mo[p, :, :, p % 8] = 1.0
    # inverse shuffle permutation matrix: perm[p_new, old(p_new)] = 1
    perm = np.zeros((P, P), dtype=np.float16)
    for s in range(4):
        for i in range(32):
            perm[32 * s + i, 32 * s + SHUF[i]] = 1.0
    return g1, g2, me.reshape(P, -1), mo.reshape(P, -1), perm


def _build_nc():
    from contextlib import ExitStack

    import concourse.bass as bass
    import concourse.mybir as mybir
    import concourse.tile as tile
    from concourse import bacc

    f32 = mybir.dt.float32
    f16 = mybir.dt.float16
    i16 = mybir.dt.int16
    u32 = mybir.dt.uint32

    nc = bacc.Bacc("TRN2", target_bir_lowering=False, debug=False)
    # in1 pre-arranged on host to [p, hb, k, g, delta*16+w2] so each weight
    # tile (k, g) is a contiguous 128-column slab (walrus requires weight
    # APs to have a single free dimension).
    in1 = nc.declare_dram_parameter(
        "in1", [P, H // RPI, CK, NT, P], f16, isOutput=False
    )
    in2 = nc.declare_dram_parameter("in2", [C, H, W], f16, isOutput=False)
    g1t = nc.declare_dram_parameter("g1t", [P, 7], i16, isOutput=False)
    g2t = nc.declare_dram_parameter("g2t", [P, G2N // 16], i16, isOutput=False)
    met = nc.declare_dram_parameter("met", [P, RD * 40], f16, isOutput=False)
    mot = nc.declare_dram_parameter("mot", [P, RD * 32], f16, isOutput=False)
    permt = nc.declare_dram_parameter("permt", [P, P], f16, isOutput=False)
    out_t = nc.declare_dram_parameter("out", [NCH, H, W], f16, isOutput=True)

    # in2 accessed (h, k, w) to match the ring's [slot, k, u] layout
    in2r = in2[:].rearrange("(k p) h w -> p h k w", p=P)

    with ExitStack() as ctx:
        tc = ctx.enter_context(tile.TileContext(nc))
        const = ctx.enter_context(tc.tile_pool(name="const", bufs=1))
        persist = ctx.enter_context(tc.tile_pool(name="persist", bufs=1))
        inp = ctx.enter_context(tc.tile_pool(name="inp", bufs=3))
        sp = ctx.enter_context(tc.tile_pool(name="sp", bufs=3))
        qp = ctx.enter_context(tc.tile_pool(name="qp", bufs=3))
        qsp = ctx.enter_context(tc.tile_pool(name="qsp", bufs=3))
        q8p = ctx.enter_context(tc.tile_pool(name="q8p", bufs=4))
        pmp = ctx.enter_context(tc.tile_pool(name="pmp", bufs=2))
        op = ctx.enter_context(tc.tile_pool(name="op", bufs=4))
        stg = ctx.enter_context(tc.tile_pool(name="stg", bufs=3))
        gram = ctx.enter_context(tc.tile_pool(name="gram", bufs=7, space="PSUM"))
        ptp = ctx.enter_context(tc.tile_pool(name="ptp", bufs=1, space="PSUM"))

        g1_s = const.tile([P, 7], i16)
        nc.scalar.dma_start(out=g1_s[:], in_=g1t[:])
        g2_s = const.tile([P, G2N // 16], i16)
        nc.scalar.dma_start(out=g2_s[:], in_=g2t[:])
        me_s = const.tile([P, RD * 40], f16)
        nc.scalar.dma_start(out=me_s[:], in_=met[:])
        mo_s = const.tile([P, RD * 32], f16)
        nc.scalar.dma_start(out=mo_s[:], in_=mot[:])
        perm_s = const.tile([P, P], f16)
        nc.scalar.dma_start(out=perm_s[:], in_=permt[:])

        # in2 ring: [p, slot, k, u]; slot(r) = (r+4) % 28, mirrored at +28
        # for slots 0..14 so the 16-slot j-window is always contiguous.
        Rr = persist.tile([P, NSLOT, CK, SROW], f16)
        # zero only what DMA never writes: the pad columns and the
        # initial rows<0 slots (0..3 and mirrors)
        nc.gpsimd.memset(Rr[:, :, :, 0:4], 0.0)
        nc.gpsimd.memset(Rr[:, :, :, 4 + W : SROW], 0.0)
        nc.gpsimd.memset(Rr[:, 0:4, :, 4 : 4 + W], 0.0)
        nc.gpsimd.memset(Rr[:, NSLOTP : NSLOTP + 4, :, 4 : 4 + W], 0.0)

        def ring_wr(s, k, r, n, eng=None):
            Rra = Rr[:]
            dst = bass.AP(
                tensor=Rra.tensor,
                offset=Rra.offset + s * (CK * SROW) + k * SROW + 4,
                ap=[Rra.ap[0], [CK * SROW, n], [1, W]],
            )
            (eng or nc.sync).dma_start(out=dst, in_=in2r[:, r : r + n, k, :])

        def load_in2_8rows(r):
            s = (r + 4) % NSLOTP
            n1 = min(8, NSLOTP - s)  # rows before wrapping past slot 27
            for k in range(CK):
                ring_wr(s, k, r, n1)
                if n1 < 8:
                    ring_wr(0, k, r + n1, 8 - n1)
                # mirror rows landing at slots 0..NMIR-1
                if s < NMIR:
                    ring_wr(s + NSLOTP, k, r, min(n1, NMIR - s))
                if n1 < 8:
                    ring_wr(NSLOTP, k, r + n1, 8 - n1)

        def zero_in2_quads(r):
            # rows r..r+7 are past the image: zero their slots (never in
            # the mirror range for our h grid)
            s = (r + 4) % NSLOTP
            nc.gpsimd.memset(Rr[:, s : s + 8, :, 4 : 4 + W], 0.0)

        win_tiles = {}

        def load_win(hh):
            t = inp.tile([P, CK, NT, P], f16, tag="win")
            nc.scalar.dma_start(out=t[:], in_=in1[:][:, hh // RPI])
            win_tiles[hh] = t

        load_in2_8rows(0)
        load_in2_8rows(8)
        load_win(0)
        load_win(8)

        def emit_output(O, h0):
            # 8 transposes into one psum tile; identity = inverse shuffle
            # permutation so columns land at true (delta, w2).
            pt = ptp.tile([P, NT, P], f16, tag="pt")
            for g in range(NT):
                nc.tensor.transpose(
                    out=pt[0:NCH, g, :],
                    in_=O[:, g * NCH : (g + 1) * NCH],
                    identity=perm_s[:],
                )
            Ost = stg.tile([NCH, RPI, W], f16)
            pta = pt[0:NCH]
            # pt[ch, g, (delta, w2)] -> Ost[ch, delta, g*16 + w2]
            src = bass.AP(
                tensor=pta.tensor,
                offset=pta.offset,
                ap=[pta.ap[0], [G, RPI], [P, NT], [1, G]],
            )
            Oa = Ost[:]
            dst = bass.AP(
                tensor=Oa.tensor,
                offset=Oa.offset,
                ap=[Oa.ap[0], [W, RPI], [G, NT], [1, G]],
            )
            nc.scalar.copy(out=dst, in_=src)
            nc.scalar.dma_start(out=out_t[:, h0 : h0 + RPI, :], in_=Ost[:])

        def run_tree(q8):
            # masked select: value for (p, rd, dxi) at q8[p, rd*16 + dxi + p%8].
            # Even dxi read q8 directly (4B-aligned); odd dxi read a +1-shifted
            # copy so their windows start even too -> everything runs 2x.
            q8a = q8[:].rearrange("p a b -> p (a b)")
            q8b = q8p.tile([P, RD * 16], f16, name="q8b", tag="q8b")
            shift_src = bass.AP(
                tensor=q8a.tensor, offset=q8a.offset + 1,
                ap=[q8a.ap[0], [1, RD * 16]],
            )
            nc.scalar.copy(out=q8b[:], in_=shift_src)

            in_e = bass.AP(
                tensor=q8a.tensor, offset=q8a.offset,
                ap=[q8a.ap[0], [16, RD], [2, 5], [1, 8]],
            )
            q8ba = q8b[:]
            in_o = bass.AP(
                tensor=q8ba.tensor, offset=q8ba.offset,
                ap=[q8ba.ap[0], [16, RD], [2, 4], [1, 8]],
            )
            pe_ = pmp.tile([P, RD, 5, 8], f16, name="pe", tag="pe")
            po_ = pmp.tile([P, RD, 4, 8], f16, name="po", tag="po")
            nc.vector.tensor_mul(
                pe_[:], in_e,
                me_s[:].rearrange("p (a b c) -> p a b c", b=5, c=8),
            )
            nc.vector.tensor_mul(
                po_[:], in_o,
                mo_s[:].rearrange("p (a b c) -> p a b c", b=4, c=8),
            )

            t1e = op.tile([P, RD, 5, 4], f16, name="t1e", tag="t1e")
            t1o = op.tile([P, RD, 4, 4], f16, name="t1o", tag="t1o")
            nc.vector.tensor_add(t1e[:], pe_[:, :, :, 0:4], pe_[:, :, :, 4:8])
            nc.vector.tensor_add(t1o[:], po_[:, :, :, 0:4], po_[:, :, :, 4:8])
            t2e = op.tile([P, RD, 5, 2], f16, name="t2e", tag="t2e")
            t2o = op.tile([P, RD, 4, 2], f16, name="t2o", tag="t2o")
            nc.vector.tensor_add(t2e[:], t1e[:, :, :, 0:2], t1e[:, :, :, 2:4])
            nc.vector.tensor_add(t2o[:], t1o[:, :, :, 0:2], t1o[:, :, :, 2:4])

            O = op.tile([P, RD * ND], f16, name="O", tag="O")
            Oa = O[:]
            out_e = bass.AP(
                tensor=Oa.tensor, offset=Oa.offset,
                ap=[Oa.ap[0], [ND, RD], [2, 5]],
            )
            out_o = bass.AP(
                tensor=Oa.tensor, offset=Oa.offset + 1,
                ap=[Oa.ap[0], [ND, RD], [2, 4]],
            )
            nc.vector.tensor_add(out_e, t2e[:, :, :, 0], t2e[:, :, :, 1])
            nc.vector.tensor_add(out_o, t2o[:, :, :, 0], t2o[:, :, :, 1])
            return O

        pend_tree = None  # (q8, h): select tree lagged one iteration
        pend_emit = []    # [(O, h)]: transposes/staging lagged two
        for h in range(0, H, RPI):
            # prefetch in2 rows h+16..h+23 (one iteration ahead)
            if h + 16 < H:
                load_in2_8rows(h + 16)
            elif h + 16 == H:
                zero_in2_quads(h + 16)
            if h + 16 < H:
                load_win(h + 16)

            win = win_tiles.pop(h)
            wa = win[:]

            # R=8 packed matmuls: per tile g, weights = 8 rows x 16 pixels
            # (p = delta*16 + w2), rhs = 16 ring rows x 24-u window, N=384.
            s0 = h % NSLOTP
            pss = []
            for g in range(NT):
                ps = gram.tile([P, BANDT], f32, tag="ps")
                for k in range(CK):
                    lhsT = bass.AP(
                        tensor=wa.tensor,
                        offset=wa.offset + (k * NT + g) * P,
                        ap=[wa.ap[0], [1, P]],
                    )
                    rhs_t = Rr[:]
                    rhs = bass.AP(
                        tensor=rhs_t.tensor,
                        offset=rhs_t.offset
                        + s0 * (CK * SROW)
                        + k * SROW
                        + G * g,
                        ap=[rhs_t.ap[0], [CK * SROW, JW], [1, UW]],
                    )
                    nc.tensor.matmul(
                        out=ps[:],
                        lhsT=lhsT,
                        rhs=rhs,
                        start=(k == 0),
                        stop=(k == CK - 1),
                    )
                pss.append(ps)

            S = sp.tile([P, NT, BANDT], f16)
            for g in range(NT):
                nc.scalar.copy(out=S[:, g, :], in_=pss[g][:])

            # gather1 -> mod-16 (whole 24-elem windows; j = delta+dy select).
            # Two half-gathers (4 tiles each, same table) so the first can
            # start as soon as the first four band copies land; data moved
            # as uint32 pairs: same 16-byte chunks, half the gpsimd cost.
            q16 = qp.tile([P, 2 * G1N, G1D], f16)
            q16f = q16[:].rearrange("p a b -> p (a b)")
            for half in range(2):
                nc.gpsimd.ap_gather(
                    out_ap=q16f[:, half * G1N * G1D : (half + 1) * G1N * G1D]
                    .bitcast(u32)
                    .rearrange("p (n d) -> p n d", d=G1D // 2),
                    in_ap=S[:, 4 * half : 4 * half + 4, :]
                    .rearrange("p a b -> p (a b)")
                    .bitcast(u32)
                    .rearrange("p (n d) -> p n d", d=G1D // 2),
                    idxs_ap=g1_s[:],
                    channels=P,
                    num_elems=NT * BANDT // (2 * G1D),
                    d=G1D // 2,
                    num_idxs=G1N,
                )

            # run the PREVIOUS iteration's select tree first: its inputs are
            # ready, so DVE stays busy while Pool finishes gather1 above.
            if pend_tree is not None:
                q8_p, h_p = pend_tree
                pend_emit.append((run_tree(q8_p), h_p))
                pend_tree = None

            qs = qsp.tile([P, RD * 24], f16)
            for half in range(2):
                nc.vector.stream_shuffle(
                    out=qs[:, half * 864 : (half + 1) * 864].bitcast(u32),
                    in_=q16f[:, half * G1N * G1D : half * G1N * G1D + 864].bitcast(
                        u32
                    ),
                    mask=SHUF,
                )

            # gather2 -> mod-8 (uint32 pairs again)
            q8 = q8p.tile([P, G2N, G1D], f16)
            nc.gpsimd.ap_gather(
                out_ap=q8[:].rearrange("p a b -> p (a b)").bitcast(u32).rearrange(
                    "p (n d) -> p n d", d=G1D // 2
                ),
                in_ap=qs[:].bitcast(u32).rearrange("p (n d) -> p n d", d=G1D // 2),
                idxs_ap=g2_s[:],
                channels=P,
                num_elems=RD * 24 // G1D,
                d=G1D // 2,
                num_idxs=G2N,
            )

            pend_tree = (q8, h)

            if len(pend_emit) >= 2:
                emit_output(*pend_emit.pop(0))

        if pend_tree is not None:
            q8_p, h_p = pend_tree
            pend_emit.append((run_tree(q8_p), h_p))
        for item in pend_emit:
            emit_output(*item)

    nc.finalize()
    return nc


def _get_nc():
    with _lock:
        if "nc" not in _cache:
            _cache["nc"] = _build_nc()
        return _cache["nc"]


def _in_maps(in1: np.ndarray, in2: np.ndarray):
    g1, g2, me, mo, perm = _host_tables()
    in1 = (np.ascontiguousarray(in1) * SCALE).astype(np.float16)
    in2 = (np.ascontiguousarray(in2) * SCALE).astype(np.float16)
    # [b, c=(k p), h=(hb delta), w=(g w2)] -> [b, p, hb, k, g, (delta w2)]
    in1 = np.ascontiguousarray(
        in1.reshape(B, CK, P, H // RPI, RPI, NT, G).transpose(0, 2, 3, 1, 5, 4, 6)
    ).reshape(B, P, H // RPI, CK, NT, P)
    return [
        {
            "in1": in1[b],
            "in2": in2[b],
            "g1t": g1,
            "g2t": g2,
            "met": me,
            "mot": mo,
            "permt": perm,
        }
        for b in range(B)
    ]


def kernel(in1: np.ndarray, in2: np.ndarray) -> np.ndarray:
    from concourse.bass_utils import run_bass_kernel_spmd

    nc = _get_nc()
    in_maps = _in_maps(in1, in2)
    res = run_bass_kernel_spmd(nc, in_maps, core_ids=list(range(B)))
    out = np.stack([res.results[b]["out"] for b in range(B)], axis=0)
    return out.astype(np.float32)


# revision 9
# speedup vs baseline: 1.1034x; 1.0242x over previous
"""Trainium2 Bass kernel for local cost-volume correlation (FlowNet-style), v3.

Problem: in1, in2 [B=8, C=256, H=96, W=128] fp32; out [B, 81, H, W] where
out[b, dy*9+dx, h, w] = mean_c in1[b,c,h,w] * in2[b,c,h+dy-4,w+dx-4] (zero pad).

Sharding: data-parallel over batch, one image per NeuronCore (8 cores).

v3 design (vs v2): R=8 row-packing, bias+max select, batched DMAs.
  - Host: in1, in2 scaled by 1/16 each (mean folded in) and cast to f16.
    Output f16, upcast on host.
  - Weight tile per (k, g): 8 in1 rows x 16 pixels (p = delta*16 + w2);
    rhs streams 16 in2 ring rows x 24-u window (N=384).  Each in2 row is
    streamed twice per k instead of 9x: PE cols/row = 2*384/8*8 = 768+tr.
  - in2 ring [P, 43, CK, 140] f16: slot(r) = (r+4) % 28, slots 0..14
    mirrored at +28 so any 16-slot window is contiguous.  One DMA per
    4-row quad (+1 mirror DMA), one iteration (8 rows) ahead.
  - Extraction per iter: psum[p=(delta,w2), j*24+u] --ACT--> S f16
    -> gather1 (3x8 chunks at g*48+(delta+dy)*3, delta = q//2... q=delta)
    -> mod-16 -> stream_shuffle (w2_3 <-> delta_0) -> gather2 (2 chunks,
    base + q%2) -> mod-8 -> ONE bias-add (0 at e==p%8 else -60000) +
    ONE max-reduce over the 8-window -> O[p, (g,dy,dx)].
  - 8 PE transposes (identity = inverse-shuffle perm), lagged one
    iteration; one ACT staging copy; one out-DMA per iter.
"""

import threading

import numpy as np

B, C, H, W = 8, 256, 96, 128
ND = 9             # displacement range per axis
NCH = ND * ND      # 81 output channels
CK = 2             # C // 128 contraction chunks
P = 128
RPI = 8            # rows per iteration
NT = 8             # pixel-group tiles per iteration (16 pixels each)
G = 16             # pixels per tile
JW = 16            # in2 row window per iteration (h-4 .. h+11)
UW = 24            # per-tile u window
BANDT = JW * UW    # 384, gram band per tile
RD = NT * ND       # 72 (g, dy) pairs per iteration
NSLOTP = 28        # physical ring slots
NMIR = 12          # slots 0..11 mirrored at +28
NSLOT = NSLOTP + NMIR  # 40
SROW = 140         # padded in2 row width (4 + 128 + 8)
G1D = 8            # gather chunk width
G1N = 112          # gather1 num_idxs per half-iteration (108 real + pad)
G2N = 160          # gather2 num_idxs (144 real + pad; pad also gives the
                   # +1-shifted q8b read one element of slack)
SCALE = 0.0625     # host-side per-input scale; SCALE^2 = 1/C

# stream_shuffle mask: swap bit3 (w2_3) with bit4 (delta_0) within each
# 32-partition quadrant
SHUF = list(range(0, 8)) + list(range(16, 24)) + list(range(8, 16)) + list(range(24, 32))

_cache = {}
_lock = threading.Lock()


def _wrap_idx(flat, ncols):
    n = flat.shape[1]
    out = np.zeros((P, ncols), dtype=np.int16)
    for q in range(8):
        for i in range(n):
            out[16 * q + (i % 16), i // 16] = flat[q, i]
    return out


def _host_tables():
    # gather1: S viewed [128, 384, 8]; for (g, dy) gather the whole 24-elem
    # u-window: 3 chunks at g*48 + (delta+dy)*3; delta = q.  -> mod-16.
    flat1 = np.zeros((8, G1N), dtype=np.int16)
    for q in range(8):
        delta = q
        for i in range(G1N):
            g, rem = divmod(min(i, 107), 27)
            dy, c = divmod(rem, 3)
            flat1[q, i] = g * 48 + (delta + dy) * 3 + c
    g1 = _wrap_idx(flat1, 7)

    # gather2: qs viewed [128, 216, 8]; for (g, dy) gather 2 chunks at
    # (g*9+dy)*3 + q%2 (q%2 = w2_3 after the shuffle).  -> mod-8.
    flat2 = np.zeros((8, G2N), dtype=np.int16)
    for q in range(8):
        for i in range(RD * 2):
            gd, c = divmod(i, 2)
            flat2[q, i] = gd * 3 + (q % 2) + c
    g2 = _wrap_idx(flat2, G2N // 16)

    # fully-materialized one-hot masks (no stride-0 dims -> DVE 2x mode)
    me = np.zeros((P, RD, 5, 8), dtype=np.float16)
    mo = np.zeros((P, RD, 4, 8), dtype=np.float16)
    for p in range(P):
        me[p, :, :, p % 8] = 1.0
        mo[p, :, :, p % 8] = 1.0
    # inverse shuffle permutation matrix: perm[p_new, old(p_new)] = 1
    perm = np.zeros((P, P), dtype=np.float16)
    for s in range(4):
        for i in range(32):
            perm[32 * s + i, 32 * s + SHUF[i]] = 1.0
    return g1, g2, me.reshape(P, -1), mo.reshape(P, -1), perm


def _build_nc():
    from contextlib import ExitStack

    import concourse.bass as bass
    import concourse.mybir as mybir
    import concourse.tile as tile
    from concourse import bacc

    f32 = mybir.dt.float32
    f16 = mybir.dt.float16
    i16 = mybir.dt.int16
    u32 = mybir.dt.uint32

    nc = bacc.Bacc("TRN2", target_bir_lowering=False, debug=False)
    # in1 pre-arranged on host to [p, hb, k, g, delta*16+w2] so each weight
    # tile (k, g) is a contiguous 128-column slab (walrus requires weight
    # APs to have a single free dimension).
    in1 = nc.declare_dram_parameter(
        "in1", [P, H // RPI, CK, NT, P], f16, isOutput=False
    )
    in2 = nc.declare_dram_parameter("in2", [C, H, W], f16, isOutput=False)
    g1t = nc.declare_dram_parameter("g1t", [P, 7], i16, isOutput=False)
    g2t = nc.declare_dram_parameter("g2t", [P, G2N // 16], i16, isOutput=False)
    met = nc.declare_dram_parameter("met", [P, RD * 40], f16, isOutput=False)
    mot = nc.declare_dram_parameter("mot", [P, RD * 32], f16, isOutput=False)
    permt = nc.declare_dram_parameter("permt", [P, P], f16, isOutput=False)
    out_t = nc.declare_dram_parameter("out", [NCH, H, W], f16, isOutput=True)

    # in2 accessed (h, k, w) to match the ring's [slot, k, u] layout
    in2r = in2[:].rearrange("(k p) h w -> p h k w", p=P)

    with ExitStack() as ctx:
        tc = ctx.enter_context(tile.TileContext(nc))
        const = ctx.enter_context(tc.tile_pool(name="const", bufs=1))
        persist = ctx.enter_context(tc.tile_pool(name="persist", bufs=1))
        inp = ctx.enter_context(tc.tile_pool(name="inp", bufs=3))
        sp = ctx.enter_context(tc.tile_pool(name="sp", bufs=3))
        qp = ctx.enter_context(tc.tile_pool(name="qp", bufs=3))
        qsp = ctx.enter_context(tc.tile_pool(name="qsp", bufs=3))
        q8p = ctx.enter_context(tc.tile_pool(name="q8p", bufs=4))
        pmp = ctx.enter_context(tc.tile_pool(name="pmp", bufs=2))
        op = ctx.enter_context(tc.tile_pool(name="op", bufs=4))
        stg = ctx.enter_context(tc.tile_pool(name="stg", bufs=3))
        gram = ctx.enter_context(tc.tile_pool(name="gram", bufs=7, space="PSUM"))
        ptp = ctx.enter_context(tc.tile_pool(name="ptp", bufs=1, space="PSUM"))

        g1_s = const.tile([P, 7], i16)
        nc.scalar.dma_start(out=g1_s[:], in_=g1t[:])
        g2_s = const.tile([P, G2N // 16], i16)
        nc.scalar.dma_start(out=g2_s[:], in_=g2t[:])
        me_s = const.tile([P, RD * 40], f16)
        nc.scalar.dma_start(out=me_s[:], in_=met[:])
        mo_s = const.tile([P, RD * 32], f16)
        nc.scalar.dma_start(out=mo_s[:], in_=mot[:])
        perm_s = const.tile([P, P], f16)
        nc.scalar.dma_start(out=perm_s[:], in_=permt[:])

        # in2 ring: [p, slot, k, u]; slot(r) = (r+4) % 28, mirrored at +28
        # for slots 0..14 so the 16-slot j-window is always contiguous.
        Rr = persist.tile([P, NSLOT, CK, SROW], f16)
        # zero only what DMA never writes: the pad columns and the
        # initial rows<0 slots (0..3 and mirrors)
        nc.gpsimd.memset(Rr[:, :, :, 0:4], 0.0)
        nc.gpsimd.memset(Rr[:, :, :, 4 + W : SROW], 0.0)
        nc.gpsimd.memset(Rr[:, 0:4, :, 4 : 4 + W], 0.0)
        nc.gpsimd.memset(Rr[:, NSLOTP : NSLOTP + 4, :, 4 : 4 + W], 0.0)

        def ring_wr(s, k, r, n, eng=None):
            Rra = Rr[:]
            dst = bass.AP(
                tensor=Rra.tensor,
                offset=Rra.offset + s * (CK * SROW) + k * SROW + 4,
                ap=[Rra.ap[0], [CK * SROW, n], [1, W]],
            )
            (eng or nc.sync).dma_start(out=dst, in_=in2r[:, r : r + n, k, :])

        def load_in2_8rows(r):
            s = (r + 4) % NSLOTP
            n1 = min(8, NSLOTP - s)  # rows before wrapping past slot 27
            for k in range(CK):
                ring_wr(s, k, r, n1)
                if n1 < 8:
                    ring_wr(0, k, r + n1, 8 - n1)
                # mirror rows landing at slots 0..NMIR-1
                if s < NMIR:
                    ring_wr(s + NSLOTP, k, r, min(n1, NMIR - s))
                if n1 < 8:
                    ring_wr(NSLOTP, k, r + n1, 8 - n1)

        def zero_in2_quads(r):
            # rows r..r+7 are past the image: zero their slots (never in
            # the mirror range for our h grid)
            s = (r + 4) % NSLOTP
            nc.gpsimd.memset(Rr[:, s : s + 8, :, 4 : 4 + W], 0.0)

        win_tiles = {}

        def load_win(hh):
            t = inp.tile([P, CK, NT, P], f16, tag="win")
            nc.sync.dma_start(out=t[:], in_=in1[:][:, hh // RPI])
            win_tiles[hh] = t

        load_in2_8rows(0)
        load_in2_8rows(8)
        load_win(0)
        load_win(8)

        def emit_output(O, h0):
            # 8 transposes into one psum tile; identity = inverse shuffle
            # permutation so columns land at true (delta, w2).
            pt = ptp.tile([P, NT, P], f16, tag="pt")
            for g in range(NT):
                nc.tensor.transpose(
                    out=pt[0:NCH, g, :],
                    in_=O[:, g * NCH : (g + 1) * NCH],
                    identity=perm_s[:],
                )
            Ost = stg.tile([NCH, RPI, W], f16)
            pta = pt[0:NCH]
            # pt[ch, g, (delta, w2)] -> Ost[ch, delta, g*16 + w2]
            src = bass.AP(
                tensor=pta.tensor,
                offset=pta.offset,
                ap=[pta.ap[0], [G, RPI], [P, NT], [1, G]],
            )
            Oa = Ost[:]
            dst = bass.AP(
                tensor=Oa.tensor,
                offset=Oa.offset,
                ap=[Oa.ap[0], [W, RPI], [G, NT], [1, G]],
            )
            nc.scalar.copy(out=dst, in_=src)
            nc.sync.dma_start(out=out_t[:, h0 : h0 + RPI, :], in_=Ost[:])

        def run_tree(q8):
            # masked select: value for (p, rd, dxi) at q8[p, rd*16 + dxi + p%8].
            # Even dxi read q8 directly (4B-aligned); odd dxi read a +1-shifted
            # copy so their windows start even too -> everything runs 2x.
            q8a = q8[:].rearrange("p a b -> p (a b)")
            q8b = q8p.tile([P, RD * 16], f16, name="q8b", tag="q8b")
            shift_src = bass.AP(
                tensor=q8a.tensor, offset=q8a.offset + 1,
                ap=[q8a.ap[0], [1, RD * 16]],
            )
            nc.scalar.copy(out=q8b[:], in_=shift_src)

            in_e = bass.AP(
                tensor=q8a.tensor, offset=q8a.offset,
                ap=[q8a.ap[0], [16, RD], [2, 5], [1, 8]],
            )
            q8ba = q8b[:]
            in_o = bass.AP(
                tensor=q8ba.tensor, offset=q8ba.offset,
                ap=[q8ba.ap[0], [16, RD], [2, 4], [1, 8]],
            )
            pe_ = pmp.tile([P, RD, 5, 8], f16, name="pe", tag="pe")
            po_ = pmp.tile([P, RD, 4, 8], f16, name="po", tag="po")
            nc.vector.tensor_mul(
                pe_[:], in_e,
                me_s[:].rearrange("p (a b c) -> p a b c", b=5, c=8),
            )
            nc.vector.tensor_mul(
                po_[:], in_o,
                mo_s[:].rearrange("p (a b c) -> p a b c", b=4, c=8),
            )

            t1e = op.tile([P, RD, 5, 4], f16, name="t1e", tag="t1e")
            t1o = op.tile([P, RD, 4, 4], f16, name="t1o", tag="t1o")
            nc.vector.tensor_add(t1e[:], pe_[:, :, :, 0:4], pe_[:, :, :, 4:8])
            nc.vector.tensor_add(t1o[:], po_[:, :, :, 0:4], po_[:, :, :, 4:8])
            t2e = op.tile([P, RD, 5, 2], f16, name="t2e", tag="t2e")
            t2o = op.tile([P, RD, 4, 2], f16, name="t2o", tag="t2o")
            nc.vector.tensor_add(t2e[:], t1e[:, :, :, 0:2], t1e[:, :, :, 2:4])
            nc.vector.tensor_add(t2o[:], t1o[:, :, :, 0:2], t1o[:, :, :, 2:4])

            O = op.tile([P, RD * ND], f16, name="O", tag="O")
            Oa = O[:]
            out_e = bass.AP(
                tensor=Oa.tensor, offset=Oa.offset,
                ap=[Oa.ap[0], [ND, RD], [2, 5]],
            )
            out_o = bass.AP(
                tensor=Oa.tensor, offset=Oa.offset + 1,
                ap=[Oa.ap[0], [ND, RD], [2, 4]],
            )
            nc.vector.tensor_add(out_e, t2e[:, :, :, 0], t2e[:, :, :, 1])
            nc.vector.tensor_add(out_o, t2o[:, :, :, 0], t2o[:, :, :, 1])
            return O

        pend_tree = None  # (q8, h): select tree lagged one iteration
        pend_emit = []    # [(O, h)]: transposes/staging lagged two
        for h in range(0, H, RPI):
            # prefetch in2 rows h+16..h+23 (one iteration ahead)
            if h + 16 < H:
                load_in2_8rows(h + 16)
            elif h + 16 == H:
                zero_in2_quads(h + 16)
            if h + 16 < H:
                load_win(h + 16)

            win = win_tiles.pop(h)
            wa = win[:]

            # R=8 packed matmuls: per tile g, weights = 8 rows x 16 pixels
            # (p = delta*16 + w2), rhs = 16 ring rows x 24-u window, N=384.
            s0 = h % NSLOTP
            pss = []
            for g in range(NT):
                ps = gram.tile([P, BANDT], f32, tag="ps")
                for k in range(CK):
                    lhsT = bass.AP(
                        tensor=wa.tensor,
                        offset=wa.offset + (k * NT + g) * P,
                        ap=[wa.ap[0], [1, P]],
                    )
                    rhs_t = Rr[:]
                    rhs = bass.AP(
                        tensor=rhs_t.tensor,
                        offset=rhs_t.offset
                        + s0 * (CK * SROW)
                        + k * SROW
                        + G * g,
                        ap=[rhs_t.ap[0], [CK * SROW, JW], [1, UW]],
                    )
                    nc.tensor.matmul(
                        out=ps[:],
                        lhsT=lhsT,
                        rhs=rhs,
                        start=(k == 0),
                        stop=(k == CK - 1),
                    )
                pss.append(ps)

            S = sp.tile([P, NT, BANDT], f16)
            for g in range(NT):
                nc.scalar.copy(out=S[:, g, :], in_=pss[g][:])

            # gather1 -> mod-16 (whole 24-elem windows; j = delta+dy select).
            # Two half-gathers (4 tiles each, same table) so the first can
            # start as soon as the first four band copies land; data moved
            # as uint32 pairs: same 16-byte chunks, half the gpsimd cost.
            q16 = qp.tile([P, 2 * G1N, G1D], f16)
            q16f = q16[:].rearrange("p a b -> p (a b)")
            for half in range(2):
                nc.gpsimd.ap_gather(
                    out_ap=q16f[:, half * G1N * G1D : (half + 1) * G1N * G1D]
                    .bitcast(u32)
                    .rearrange("p (n d) -> p n d", d=G1D // 2),
                    in_ap=S[:, 4 * half : 4 * half + 4, :]
                    .rearrange("p a b -> p (a b)")
                    .bitcast(u32)
                    .rearrange("p (n d) -> p n d", d=G1D // 2),
                    idxs_ap=g1_s[:],
                    channels=P,
                    num_elems=NT * BANDT // (2 * G1D),
                    d=G1D // 2,
                    num_idxs=G1N,
                )

            # run the PREVIOUS iteration's select tree first: its inputs are
            # ready, so DVE stays busy while Pool finishes gather1 above.
            if pend_tree is not None:
                q8_p, h_p = pend_tree
                pend_emit.append((run_tree(q8_p), h_p))
                pend_tree = None

            qs = qsp.tile([P, RD * 24], f16)
            for half in range(2):
                nc.vector.stream_shuffle(
                    out=qs[:, half * 864 : (half + 1) * 864].bitcast(u32),
                    in_=q16f[:, half * G1N * G1D : half * G1N * G1D + 864].bitcast(
                        u32
                    ),
                    mask=SHUF,
                )

            # gather2 -> mod-8 (uint32 pairs again)
            q8 = q8p.tile([P, G2N, G1D], f16)
            nc.gpsimd.ap_gather(
                out_ap=q8[:].rearrange("p a b -> p (a b)").bitcast(u32).rearrange(
                    "p (n d) -> p n d", d=G1D // 2
                ),
                in_ap=qs[:].bitcast(u32).rearrange("p (n d) -> p n d", d=G1D // 2),
                idxs_ap=g2_s[:],
                channels=P,
                num_elems=RD * 24 // G1D,
                d=G1D // 2,
                num_idxs=G2N,
            )

            pend_tree = (q8, h)

            if len(pend_emit) >= 2:
                emit_output(*pend_emit.pop(0))

        if pend_tree is not None:
            q8_p, h_p = pend_tree
            pend_emit.append((run_tree(q8_p), h_p))
        for item in pend_emit:
            emit_output(*item)

    nc.finalize()
    return nc


def _get_nc():
    with _lock:
        if "nc" not in _cache:
            _cache["nc"] = _build_nc()
        return _cache["nc"]


def _in_maps(in1: np.ndarray, in2: np.ndarray):
    g1, g2, me, mo, perm = _host_tables()
    in1 = (np.ascontiguousarray(in1) * SCALE).astype(np.float16)
    in2 = (np.ascontiguousarray(in2) * SCALE).astype(np.float16)
    # [b, c=(k p), h=(hb delta), w=(g w2)] -> [b, p, hb, k, g, (delta w2)]
    in1 = np.ascontiguousarray(
        in1.reshape(B, CK, P, H // RPI, RPI, NT, G).transpose(0, 2, 3, 1, 5, 4, 6)
    ).reshape(B, P, H // RPI, CK, NT, P)
    return [
        {
            "in1": in1[b],
            "in2": in2[b],
            "g1t": g1,
            "g2t": g2,
            "met": me,
            "mot": mo,
            "permt": perm,
        }
        for b in range(B)
    ]


def kernel(in1: np.ndarray, in2: np.ndarray) -> np.ndarray:
    from concourse.bass_utils import run_bass_kernel_spmd

    nc = _get_nc()
    in_maps = _in_maps(in1, in2)
    res = run_bass_kernel_spmd(nc, in_maps, core_ids=list(range(B)))
    out = np.stack([res.results[b]["out"] for b in range(B)], axis=0)
    return out.astype(np.float32)


# revision 10
# speedup vs baseline: 1.1241x; 1.0187x over previous
"""Trainium2 Bass kernel for local cost-volume correlation (FlowNet-style), v3.

Problem: in1, in2 [B=8, C=256, H=96, W=128] fp32; out [B, 81, H, W] where
out[b, dy*9+dx, h, w] = mean_c in1[b,c,h,w] * in2[b,c,h+dy-4,w+dx-4] (zero pad).

Sharding: data-parallel over batch, one image per NeuronCore (8 cores).

v3 design (vs v2): R=8 row-packing, bias+max select, batched DMAs.
  - Host: in1, in2 scaled by 1/16 each (mean folded in) and cast to f16.
    Output f16, upcast on host.
  - Weight tile per (k, g): 8 in1 rows x 16 pixels (p = delta*16 + w2);
    rhs streams 16 in2 ring rows x 24-u window (N=384).  Each in2 row is
    streamed twice per k instead of 9x: PE cols/row = 2*384/8*8 = 768+tr.
  - in2 ring [P, 43, CK, 140] f16: slot(r) = (r+4) % 28, slots 0..14
    mirrored at +28 so any 16-slot window is contiguous.  One DMA per
    4-row quad (+1 mirror DMA), one iteration (8 rows) ahead.
  - Extraction per iter: psum[p=(delta,w2), j*24+u] --ACT--> S f16
    -> gather1 (3x8 chunks at g*48+(delta+dy)*3, delta = q//2... q=delta)
    -> mod-16 -> stream_shuffle (w2_3 <-> delta_0) -> gather2 (2 chunks,
    base + q%2) -> mod-8 -> ONE bias-add (0 at e==p%8 else -60000) +
    ONE max-reduce over the 8-window -> O[p, (g,dy,dx)].
  - 8 PE transposes (identity = inverse-shuffle perm), lagged one
    iteration; one ACT staging copy; one out-DMA per iter.
"""

import threading

import numpy as np

B, C, H, W = 8, 256, 96, 128
ND = 9             # displacement range per axis
NCH = ND * ND      # 81 output channels
CK = 2             # C // 128 contraction chunks
P = 128
RPI = 8            # rows per iteration
NT = 8             # pixel-group tiles per iteration (16 pixels each)
G = 16             # pixels per tile
JW = 16            # in2 row window per iteration (h-4 .. h+11)
UW = 24            # per-tile u window
BANDT = JW * UW    # 384, gram band per tile
RD = NT * ND       # 72 (g, dy) pairs per iteration
NSLOTP = 28        # physical ring slots
NMIR = 12          # slots 0..11 mirrored at +28
NSLOT = NSLOTP + NMIR  # 40
SROW = 140         # padded in2 row width (4 + 128 + 8)
G1D = 8            # gather chunk width
G1N = 112          # gather1 num_idxs per half-iteration (108 real + pad)
G2N = 160          # gather2 num_idxs (144 real + pad; pad also gives the
                   # +1-shifted q8b read one element of slack)
SCALE = 0.0625     # host-side per-input scale; SCALE^2 = 1/C

# stream_shuffle mask: swap bit3 (w2_3) with bit4 (delta_0) within each
# 32-partition quadrant
SHUF = list(range(0, 8)) + list(range(16, 24)) + list(range(8, 16)) + list(range(24, 32))

_cache = {}
_lock = threading.Lock()


def _wrap_idx(flat, ncols):
    n = flat.shape[1]
    out = np.zeros((P, ncols), dtype=np.int16)
    for q in range(8):
        for i in range(n):
            out[16 * q + (i % 16), i // 16] = flat[q, i]
    return out


def _host_tables():
    # gather1: S viewed [128, 384, 8]; for (g, dy) gather the whole 24-elem
    # u-window: 3 chunks at g*48 + (delta+dy)*3; delta = q.  -> mod-16.
    flat1 = np.zeros((8, G1N), dtype=np.int16)
    for q in range(8):
        delta = q
        for i in range(G1N):
            g, rem = divmod(min(i, 107), 27)
            dy, c = divmod(rem, 3)
            flat1[q, i] = g * 48 + (delta + dy) * 3 + c
    g1 = _wrap_idx(flat1, 7)

    # gather2: qs viewed [128, 216, 8]; for (g, dy) gather 2 chunks at
    # (g*9+dy)*3 + q%2 (q%2 = w2_3 after the shuffle).  -> mod-8.
    flat2 = np.zeros((8, G2N), dtype=np.int16)
    for q in range(8):
        for i in range(RD * 2):
            gd, c = divmod(i, 2)
            flat2[q, i] = gd * 3 + (q % 2) + c
    g2 = _wrap_idx(flat2, G2N // 16)

    # fully-materialized one-hot masks (no stride-0 dims -> DVE 2x mode)
    me = np.zeros((P, RD, 5, 8), dtype=np.float16)
    mo = np.zeros((P, RD, 4, 8), dtype=np.float16)
    for p in range(P):
        me[p, :, :, p % 8] = 1.0
        mo[p, :, :, p % 8] = 1.0
    # inverse shuffle permutation matrix: perm[p_new, old(p_new)] = 1
    perm = np.zeros((P, P), dtype=np.float16)
    for s in range(4):
        for i in range(32):
            perm[32 * s + i, 32 * s + SHUF[i]] = 1.0
    return g1, g2, me.reshape(P, -1), mo.reshape(P, -1), perm


def _build_nc():
    from contextlib import ExitStack

    import concourse.bass as bass
    import concourse.mybir as mybir
    import concourse.tile as tile
    from concourse import bacc

    f32 = mybir.dt.float32
    f16 = mybir.dt.float16
    i16 = mybir.dt.int16
    u32 = mybir.dt.uint32

    nc = bacc.Bacc("TRN2", target_bir_lowering=False, debug=False)
    # in1 pre-arranged on host to [p, hb, k, g, delta*16+w2] so each weight
    # tile (k, g) is a contiguous 128-column slab (walrus requires weight
    # APs to have a single free dimension).
    in1 = nc.declare_dram_parameter(
        "in1", [P, H // RPI, CK, NT, P], f16, isOutput=False
    )
    in2 = nc.declare_dram_parameter("in2", [C, H, W], f16, isOutput=False)
    g1t = nc.declare_dram_parameter("g1t", [P, 7], i16, isOutput=False)
    g2t = nc.declare_dram_parameter("g2t", [P, G2N // 16], i16, isOutput=False)
    met = nc.declare_dram_parameter("met", [P, RD * 40], f16, isOutput=False)
    mot = nc.declare_dram_parameter("mot", [P, RD * 32], f16, isOutput=False)
    permt = nc.declare_dram_parameter("permt", [P, P], f16, isOutput=False)
    out_t = nc.declare_dram_parameter("out", [NCH, H, W], f16, isOutput=True)

    # in2 accessed (h, k, w) to match the ring's [slot, k, u] layout
    in2r = in2[:].rearrange("(k p) h w -> p h k w", p=P)

    with ExitStack() as ctx:
        tc = ctx.enter_context(tile.TileContext(nc))
        const = ctx.enter_context(tc.tile_pool(name="const", bufs=1))
        persist = ctx.enter_context(tc.tile_pool(name="persist", bufs=1))
        inp = ctx.enter_context(tc.tile_pool(name="inp", bufs=3))
        sp = ctx.enter_context(tc.tile_pool(name="sp", bufs=3))
        qp = ctx.enter_context(tc.tile_pool(name="qp", bufs=3))
        qsp = ctx.enter_context(tc.tile_pool(name="qsp", bufs=3))
        q8p = ctx.enter_context(tc.tile_pool(name="q8p", bufs=4))
        pmp = ctx.enter_context(tc.tile_pool(name="pmp", bufs=2))
        op = ctx.enter_context(tc.tile_pool(name="op", bufs=4))
        stg = ctx.enter_context(tc.tile_pool(name="stg", bufs=3))
        gram = ctx.enter_context(tc.tile_pool(name="gram", bufs=7, space="PSUM"))
        ptp = ctx.enter_context(tc.tile_pool(name="ptp", bufs=1, space="PSUM"))

        g1_s = const.tile([P, 7], i16)
        nc.scalar.dma_start(out=g1_s[:], in_=g1t[:])
        g2_s = const.tile([P, G2N // 16], i16)
        nc.scalar.dma_start(out=g2_s[:], in_=g2t[:])
        me_s = const.tile([P, RD * 40], f16)
        nc.scalar.dma_start(out=me_s[:], in_=met[:])
        mo_s = const.tile([P, RD * 32], f16)
        nc.scalar.dma_start(out=mo_s[:], in_=mot[:])
        perm_s = const.tile([P, P], f16)
        nc.scalar.dma_start(out=perm_s[:], in_=permt[:])

        # in2 ring: [p, slot, k, u]; slot(r) = (r+4) % 28, mirrored at +28
        # for slots 0..14 so the 16-slot j-window is always contiguous.
        Rr = persist.tile([P, NSLOT, CK, SROW], f16)
        # zero only what DMA never writes: the pad columns and the
        # initial rows<0 slots (0..3 and mirrors)
        nc.gpsimd.memset(Rr[:, :, :, 0:4], 0.0)
        nc.gpsimd.memset(Rr[:, :, :, 4 + W : SROW], 0.0)
        nc.gpsimd.memset(Rr[:, 0:4, :, 4 : 4 + W], 0.0)
        nc.gpsimd.memset(Rr[:, NSLOTP : NSLOTP + 4, :, 4 : 4 + W], 0.0)

        def ring_wr(s, k, r, n, eng=None):
            Rra = Rr[:]
            dst = bass.AP(
                tensor=Rra.tensor,
                offset=Rra.offset + s * (CK * SROW) + k * SROW + 4,
                ap=[Rra.ap[0], [CK * SROW, n], [1, W]],
            )
            (eng or nc.sync).dma_start(out=dst, in_=in2r[:, r : r + n, k, :])

        def load_in2_8rows(r):
            s = (r + 4) % NSLOTP
            n1 = min(8, NSLOTP - s)  # rows before wrapping past slot 27
            for k in range(CK):
                ring_wr(s, k, r, n1)
                if n1 < 8:
                    ring_wr(0, k, r + n1, 8 - n1)
                # mirror rows landing at slots 0..NMIR-1
                if s < NMIR:
                    ring_wr(s + NSLOTP, k, r, min(n1, NMIR - s))
                if n1 < 8:
                    ring_wr(NSLOTP, k, r + n1, 8 - n1)

        def zero_in2_quads(r):
            # rows r..r+7 are past the image: zero their slots (never in
            # the mirror range for our h grid)
            s = (r + 4) % NSLOTP
            nc.gpsimd.memset(Rr[:, s : s + 8, :, 4 : 4 + W], 0.0)

        win_tiles = {}

        def load_win(hh):
            t = inp.tile([P, CK, NT, P], f16, tag="win")
            nc.sync.dma_start(out=t[:], in_=in1[:][:, hh // RPI])
            win_tiles[hh] = t

        load_in2_8rows(0)
        load_in2_8rows(8)
        load_win(0)
        load_win(8)

        def emit_output(O, h0):
            # 8 transposes into one psum tile; identity = inverse shuffle
            # permutation so columns land at true (delta, w2).
            pt = ptp.tile([P, NT, P], f16, tag="pt")
            for g in range(NT):
                nc.tensor.transpose(
                    out=pt[0:NCH, g, :],
                    in_=O[:, g * NCH : (g + 1) * NCH],
                    identity=perm_s[:],
                )
            Ost = stg.tile([NCH, RPI, W], f16)
            pta = pt[0:NCH]
            # pt[ch, g, (delta, w2)] -> Ost[ch, delta, g*16 + w2]
            src = bass.AP(
                tensor=pta.tensor,
                offset=pta.offset,
                ap=[pta.ap[0], [G, RPI], [P, NT], [1, G]],
            )
            Oa = Ost[:]
            dst = bass.AP(
                tensor=Oa.tensor,
                offset=Oa.offset,
                ap=[Oa.ap[0], [W, RPI], [G, NT], [1, G]],
            )
            nc.scalar.copy(out=dst, in_=src)
            nc.sync.dma_start(out=out_t[:, h0 : h0 + RPI, :], in_=Ost[:])

        def run_tree(q8):
            # masked select: value for (p, rd, dxi) at q8[p, rd*16 + dxi + p%8].
            # Even dxi read q8 directly (4B-aligned); odd dxi read a +1-shifted
            # copy so their windows start even too -> everything runs 2x.
            q8a = q8[:].rearrange("p a b -> p (a b)")
            q8b = q8p.tile([P, RD * 16], f16, name="q8b", tag="q8b")
            shift_src = bass.AP(
                tensor=q8a.tensor, offset=q8a.offset + 1,
                ap=[q8a.ap[0], [1, RD * 16]],
            )
            nc.scalar.copy(out=q8b[:], in_=shift_src)

            in_e = bass.AP(
                tensor=q8a.tensor, offset=q8a.offset,
                ap=[q8a.ap[0], [16, RD], [2, 5], [1, 8]],
            )
            q8ba = q8b[:]
            in_o = bass.AP(
                tensor=q8ba.tensor, offset=q8ba.offset,
                ap=[q8ba.ap[0], [16, RD], [2, 4], [1, 8]],
            )
            pe_ = pmp.tile([P, RD, 5, 8], f16, name="pe", tag="pe")
            po_ = pmp.tile([P, RD, 4, 8], f16, name="po", tag="po")
            nc.vector.tensor_mul(
                pe_[:], in_e,
                me_s[:].rearrange("p (a b c) -> p a b c", b=5, c=8),
            )
            nc.vector.tensor_mul(
                po_[:], in_o,
                mo_s[:].rearrange("p (a b c) -> p a b c", b=4, c=8),
            )

            t1e = op.tile([P, RD, 5, 4], f16, name="t1e", tag="t1e")
            t1o = op.tile([P, RD, 4, 4], f16, name="t1o", tag="t1o")
            nc.vector.tensor_add(t1e[:], pe_[:, :, :, 0:4], pe_[:, :, :, 4:8])
            nc.vector.tensor_add(t1o[:], po_[:, :, :, 0:4], po_[:, :, :, 4:8])
            t2e = op.tile([P, RD, 5, 2], f16, name="t2e", tag="t2e")
            t2o = op.tile([P, RD, 4, 2], f16, name="t2o", tag="t2o")
            nc.vector.tensor_add(t2e[:], t1e[:, :, :, 0:2], t1e[:, :, :, 2:4])
            nc.vector.tensor_add(t2o[:], t1o[:, :, :, 0:2], t1o[:, :, :, 2:4])

            O = op.tile([P, RD * ND], f16, name="O", tag="O")
            Oa = O[:]
            out_e = bass.AP(
                tensor=Oa.tensor, offset=Oa.offset,
                ap=[Oa.ap[0], [ND, RD], [2, 5]],
            )
            out_o = bass.AP(
                tensor=Oa.tensor, offset=Oa.offset + 1,
                ap=[Oa.ap[0], [ND, RD], [2, 4]],
            )
            nc.vector.tensor_add(out_e, t2e[:, :, :, 0], t2e[:, :, :, 1])
            nc.vector.tensor_add(out_o, t2o[:, :, :, 0], t2o[:, :, :, 1])
            return O

        pend_tree = []  # (q8, h): select tree lagged one iteration
        pend_emit = []    # [(O, h)]: transposes/staging lagged two
        for h in range(0, H, RPI):
            # prefetch in2 rows h+16..h+23 (one iteration ahead)
            if h + 16 < H:
                load_in2_8rows(h + 16)
            elif h + 16 == H:
                zero_in2_quads(h + 16)
            if h + 16 < H:
                load_win(h + 16)

            win = win_tiles.pop(h)
            wa = win[:]

            # R=8 packed matmuls: per tile g, weights = 8 rows x 16 pixels
            # (p = delta*16 + w2), rhs = 16 ring rows x 24-u window, N=384.
            s0 = h % NSLOTP
            pss = []
            for g in range(NT):
                ps = gram.tile([P, BANDT], f32, tag="ps")
                for k in range(CK):
                    lhsT = bass.AP(
                        tensor=wa.tensor,
                        offset=wa.offset + (k * NT + g) * P,
                        ap=[wa.ap[0], [1, P]],
                    )
                    rhs_t = Rr[:]
                    rhs = bass.AP(
                        tensor=rhs_t.tensor,
                        offset=rhs_t.offset
                        + s0 * (CK * SROW)
                        + k * SROW
                        + G * g,
                        ap=[rhs_t.ap[0], [CK * SROW, JW], [1, UW]],
                    )
                    nc.tensor.matmul(
                        out=ps[:],
                        lhsT=lhsT,
                        rhs=rhs,
                        start=(k == 0),
                        stop=(k == CK - 1),
                    )
                pss.append(ps)

            S = sp.tile([P, NT, BANDT], f16)
            for g in range(NT):
                nc.scalar.copy(out=S[:, g, :], in_=pss[g][:])

            # gather1 -> mod-16 (whole 24-elem windows; j = delta+dy select).
            # Two half-gathers (4 tiles each, same table) so the first can
            # start as soon as the first four band copies land; data moved
            # as uint32 pairs: same 16-byte chunks, half the gpsimd cost.
            q16 = qp.tile([P, 2 * G1N, G1D], f16)
            q16f = q16[:].rearrange("p a b -> p (a b)")
            for half in range(2):
                nc.gpsimd.ap_gather(
                    out_ap=q16f[:, half * G1N * G1D : (half + 1) * G1N * G1D]
                    .bitcast(u32)
                    .rearrange("p (n d) -> p n d", d=G1D // 2),
                    in_ap=S[:, 4 * half : 4 * half + 4, :]
                    .rearrange("p a b -> p (a b)")
                    .bitcast(u32)
                    .rearrange("p (n d) -> p n d", d=G1D // 2),
                    idxs_ap=g1_s[:],
                    channels=P,
                    num_elems=NT * BANDT // (2 * G1D),
                    d=G1D // 2,
                    num_idxs=G1N,
                )

            # run the PREVIOUS iteration's select tree first: its inputs are
            # ready, so DVE stays busy while Pool finishes gather1 above.
            if len(pend_tree) >= 2:
                q8_p, h_p = pend_tree.pop(0)
                pend_emit.append((run_tree(q8_p), h_p))

            qs = qsp.tile([P, RD * 24], f16)
            for half in range(2):
                nc.vector.stream_shuffle(
                    out=qs[:, half * 864 : (half + 1) * 864].bitcast(u32),
                    in_=q16f[:, half * G1N * G1D : half * G1N * G1D + 864].bitcast(
                        u32
                    ),
                    mask=SHUF,
                )

            # gather2 -> mod-8 (uint32 pairs again)
            q8 = q8p.tile([P, G2N, G1D], f16)
            nc.gpsimd.ap_gather(
                out_ap=q8[:].rearrange("p a b -> p (a b)").bitcast(u32).rearrange(
                    "p (n d) -> p n d", d=G1D // 2
                ),
                in_ap=qs[:].bitcast(u32).rearrange("p (n d) -> p n d", d=G1D // 2),
                idxs_ap=g2_s[:],
                channels=P,
                num_elems=RD * 24 // G1D,
                d=G1D // 2,
                num_idxs=G2N,
            )

            pend_tree.append((q8, h))

            if len(pend_emit) >= 2:
                emit_output(*pend_emit.pop(0))

        for q8_p, h_p in pend_tree:
            pend_emit.append((run_tree(q8_p), h_p))
        for item in pend_emit:
            emit_output(*item)

    nc.finalize()
    return nc


def _get_nc():
    with _lock:
        if "nc" not in _cache:
            _cache["nc"] = _build_nc()
        return _cache["nc"]


def _in_maps(in1: np.ndarray, in2: np.ndarray):
    g1, g2, me, mo, perm = _host_tables()
    in1 = (np.ascontiguousarray(in1) * SCALE).astype(np.float16)
    in2 = (np.ascontiguousarray(in2) * SCALE).astype(np.float16)
    # [b, c=(k p), h=(hb delta), w=(g w2)] -> [b, p, hb, k, g, (delta w2)]
    in1 = np.ascontiguousarray(
        in1.reshape(B, CK, P, H // RPI, RPI, NT, G).transpose(0, 2, 3, 1, 5, 4, 6)
    ).reshape(B, P, H // RPI, CK, NT, P)
    return [
        {
            "in1": in1[b],
            "in2": in2[b],
            "g1t": g1,
            "g2t": g2,
            "met": me,
            "mot": mo,
            "permt": perm,
        }
        for b in range(B)
    ]


def kernel(in1: np.ndarray, in2: np.ndarray) -> np.ndarray:
    from concourse.bass_utils import run_bass_kernel_spmd

    nc = _get_nc()
    in_maps = _in_maps(in1, in2)
    res = run_bass_kernel_spmd(nc, in_maps, core_ids=list(range(B)))
    out = np.stack([res.results[b]["out"] for b in range(B)], axis=0)
    return out.astype(np.float32)
